# revision 1
# baseline (speedup 1.0000x reference)
"""CFG dual cross-attention on 8 Trainium2 NeuronCores (Bass/Tile).

Sharding: the cfg axis (cond/uncond) splits the 8 cores into 2 groups of 4;
within a group the 4096 query rows are sharded 4-way (1024 rows/core).  Every
core computes the full K/V projection of its group's context (replicated
within the group) and runs all 40 heads for its own query rows, so there are
no cross-core collectives; the host just concatenates the row shards.

All weights are transposed on the host so every matmul contracts over the
SBUF partition dim.  Matmul operands are bf16 (fp32 PSUM accumulation);
softmax/rms statistics are computed in fp32.

Attention uses a transposed-logits formulation: logits come out of the PE as
[L-partition, s-free] per head, so exp (no max subtraction needed — q/k are
rms-normalized, |logit| < ~8 is far inside fp32/exp range), the key-axis sum
(ones-matmul), and the A@V contraction all happen without any transposes.
The 1/sum normalizer is replicated across partitions with a rank-1 float32r
matmul and folded into the per-head output eviction.
"""

from contextlib import ExitStack

import numpy as np

import concourse.bass as bass
import concourse.bacc as bacc
import concourse.mybir as mybir
import concourse.tile as tile
from concourse import bass_utils

EPS = 1e-6
F32 = mybir.dt.float32
F32R = mybir.dt.float32r

# ---- problem shape (nn_CFGDualCrossAttention: D=5120, H=40, S=4096, L=512) ----
D = 5120
L = 512
S_SHARD = 1024        # 4096 / 4 cores per cfg group
S_CHUNK = 256
MM = mybir.dt.bfloat16

TRACE = False         # set by test harness for NTFF timing
LAST_EXEC_NS = None
_CACHED_NC = None


def _build() -> bacc.Bacc:
    KO = D // 128         # contraction subtiles
    H = D // 128          # heads (head_dim 128)
    LSUB = L // 128
    CH = S_SHARD // S_CHUNK
    SCALE = float(128 ** -0.5)
    VN = min(D, 256)      # v projection free-dim stripe
    ON = min(D, 512)      # output projection free-dim stripe
    mm = MM
    SCH = S_CHUNK
    # how many [128, SCH] fp32 sub-tiles share one 2 KB PSUM bank (zero-region)
    LB_PER_BANK = max(1, 512 // SCH)

    nc = bacc.Bacc("TRN2", target_bir_lowering=False, debug=False, num_devices=8)

    hT = nc.dram_tensor("hT", [D, S_SHARD], mm, kind="ExternalInput")
    cT = nc.dram_tensor("cT", [D, L], mm, kind="ExternalInput")
    wqT = nc.dram_tensor("wqT", [D, D], mm, kind="ExternalInput")
    wkT = nc.dram_tensor("wkT", [D, D], mm, kind="ExternalInput")
    wvT = nc.dram_tensor("wvT", [D, D], mm, kind="ExternalInput")
    woT = nc.dram_tensor("woT", [D, D], mm, kind="ExternalInput")
    bqt = nc.dram_tensor("bq", [D], F32, kind="ExternalInput")
    bkt = nc.dram_tensor("bk", [D], F32, kind="ExternalInput")
    bvt = nc.dram_tensor("bv", [D], F32, kind="ExternalInput")
    bot = nc.dram_tensor("bo", [D], F32, kind="ExternalInput")
    gqt = nc.dram_tensor("gq", [D], F32, kind="ExternalInput")
    gkt = nc.dram_tensor("gk", [D], F32, kind="ExternalInput")
    out = nc.dram_tensor("out", [S_SHARD, D], F32, kind="ExternalOutput")
    oT_dram = nc.dram_tensor("oT_spill", [D, S_SHARD], mm)

    hT_r = hT.rearrange("(ko p) s -> p ko s", p=128)
    cT_r = cT.rearrange("(ko p) l -> p ko l", p=128)
    wqT_r = wqT.rearrange("(ko p) n -> p ko n", p=128)
    wkT_r = wkT.rearrange("(ko p) n -> p ko n", p=128)
    wvT_r = wvT.rearrange("(ko p) n -> p ko n", p=128)
    woT_r = woT.rearrange("(ko p) n -> p ko n", p=128)
    oT_r = oT_dram.rearrange("(ko p) s -> p ko s", p=128)
    out_r = out.rearrange("(cs p) n -> p cs n", p=128)

    with tile.TileContext(nc) as tc, ExitStack() as top:
        consts = top.enter_context(tc.tile_pool(name="consts", bufs=1))
        bq_sb = consts.tile([128, KO], F32)
        bk_sb = consts.tile([128, KO], F32)
        gq_sb = consts.tile([128, KO], F32)
        gk_sb = consts.tile([128, KO], F32)
        ones_sb = consts.tile([128, 1], mm)
        ones_row = consts.tile([1, 128], F32R)
        eps_sb = consts.tile([128, 1], F32)
        nc.sync.dma_start(bq_sb, bqt.rearrange("(m p) -> p m", p=128))
        nc.sync.dma_start(bk_sb, bkt.rearrange("(m p) -> p m", p=128))
        nc.sync.dma_start(gq_sb, gqt.rearrange("(m p) -> p m", p=128))
        nc.sync.dma_start(gk_sb, gkt.rearrange("(m p) -> p m", p=128))
        ones_row_f = consts.tile([1, 128], F32)
        nc.vector.memset(ones_sb, 1.0)
        nc.vector.memset(ones_row_f, 1.0)
        with nc.allow_low_precision(reason="f32r rounding of constant ones"):
            nc.vector.tensor_copy(ones_row, ones_row_f)
        nc.vector.memset(eps_sb, EPS)

        # k.T and v live across KV + attention phases; released before O proj
        with ExitStack() as acts_scope:
            act_pool = acts_scope.enter_context(tc.tile_pool(name="acts", bufs=1))
            kT_sb = act_pool.tile([128, KO, L], mm)
            v_sb = act_pool.tile([128, LSUB, D], mm)

            # =========== K + V (context projections) ===========
            with ExitStack() as ph:
                cpool = ph.enter_context(tc.tile_pool(name="ctx", bufs=1))
                wpool = ph.enter_context(tc.tile_pool(name="wkv", bufs=3))
                spool = ph.enter_context(tc.tile_pool(name="scratch", bufs=2))
                pp_mm = ph.enter_context(tc.tile_pool(name="ppmm", bufs=2, space="PSUM"))
                pp_ss = ph.enter_context(tc.tile_pool(name="ppss", bufs=1, space="PSUM"))

                cT_sb = cpool.tile([128, KO, L], mm)
                nc.sync.dma_start(cT_sb, cT_r)
                bv_rep = cpool.tile([128, D], mm, name="bv_rep")
                nc.gpsimd.dma_start(bv_rep, bvt.ap()[None, :].to_broadcast([128, D]))

                ss_ps = pp_ss.tile([128, 512], F32, name="ps_ss")
                for mp in range(KO // 2):
                    wk_sb = wpool.tile([128, KO, 256], mm, tag="w")
                    nc.sync.dma_start(wk_sb, wkT_r[:, :, bass.ts(mp, 256)])
                    for mi in range(2):
                        m = 2 * mp + mi
                        ps = pp_mm.tile([128, 512], F32, tag="mm",
                                        name="ps_mm")[:, :L]
                        for ko in range(KO):
                            nc.tensor.matmul(ps, wk_sb[:, ko, bass.ts(mi, 128)],
                                             cT_sb[:, ko, :],
                                             start=(ko == 0), stop=(ko == KO - 1))
                        nc.scalar.activation(kT_sb[:, m, :], ps,
                                             mybir.ActivationFunctionType.Identity,
                                             bias=bk_sb[:, m:m + 1])
                        sq = spool.tile([128, 512], mm, tag="sq",
                                        name="sq")[:, :L]
                        nc.vector.tensor_mul(sq, kT_sb[:, m, :], kT_sb[:, m, :])
                        nc.tensor.matmul(ss_ps[:1, :L], ones_sb, sq,
                                         start=(m == 0), stop=(m == KO - 1))
                kinv_sb = cpool.tile([1, L], F32, name="kinv")
                nc.scalar.activation(kinv_sb, ss_ps[:1, :L],
                                     mybir.ActivationFunctionType.Sqrt,
                                     scale=1.0 / D, bias=eps_sb[:1])
                nc.vector.reciprocal(kinv_sb, kinv_sb)
                kinv_rep = cpool.tile([128, L], F32, name="kinv_rep")
                nc.gpsimd.partition_broadcast(kinv_rep, kinv_sb)
                for m in range(KO):
                    nc.vector.scalar_tensor_tensor(
                        out=kT_sb[:, m, :], in0=kT_sb[:, m, :],
                        scalar=gk_sb[:, m:m + 1], in1=kinv_rep,
                        op0=mybir.AluOpType.mult, op1=mybir.AluOpType.mult)

                for n in range(D // VN):
                    wv_sb = wpool.tile([128, KO, VN], mm, tag="w", name="wv_sb")
                    nc.sync.dma_start(wv_sb, wvT_r[:, :, bass.ts(n, VN)])
                    for lb in range(LSUB):
                        ps = pp_mm.tile([128, 512], F32, tag="mm",
                                        name="ps_mm")[:, :VN]
                        for ko in range(KO):
                            nc.tensor.matmul(ps, cT_sb[:, ko, bass.ts(lb, 128)],
                                             wv_sb[:, ko, :],
                                             start=(ko == 0), stop=(ko == KO - 1))
                        nc.vector.tensor_add(v_sb[:, lb, bass.ts(n, VN)], ps,
                                             bv_rep[:, bass.ts(n, VN)])

            # =========== Q projection + attention, per seq chunk ===========
            for sc in range(CH):
                with ExitStack() as ch_scope:
                    qpool = ch_scope.enter_context(tc.tile_pool(name="qch", bufs=1))
                    spool = ch_scope.enter_context(tc.tile_pool(name="qscr", bufs=2))
                    apool = ch_scope.enter_context(tc.tile_pool(name="attn", bufs=2))
                    opool = ch_scope.enter_context(tc.tile_pool(name="oev", bufs=3))
                    pp_ss = ch_scope.enter_context(tc.tile_pool(name="ppss", bufs=1, space="PSUM"))
                    pp_pt = ch_scope.enter_context(tc.tile_pool(name="pppt", bufs=1, space="PSUM"))
                    pp_r = ch_scope.enter_context(tc.tile_pool(name="ppr", bufs=1, space="PSUM"))
                    pp_o = ch_scope.enter_context(tc.tile_pool(name="ppo", bufs=2, space="PSUM"))

                    qT_sb = qpool.tile([128, KO, SCH], mm)
                    qsc_rep = qpool.tile([128, SCH], F32, name="qsc_rep")

                    # ---- q.T chunk = Wq @ hT (+bq), rms stats in fp32 ----
                    with ExitStack() as qproj:
                        hpool = qproj.enter_context(tc.tile_pool(name="hq", bufs=1))
                        wpool = qproj.enter_context(tc.tile_pool(name="wq", bufs=3))
                        pp_mm = qproj.enter_context(tc.tile_pool(name="ppmmq", bufs=2, space="PSUM"))

                        hT_sb = hpool.tile([128, KO, SCH], mm)
                        nc.sync.dma_start(hT_sb, hT_r[:, :, bass.ts(sc, SCH)])
                        ss_ps = pp_ss.tile([128, 512], F32, tag="ss",
                                           name="ps_ss")[:1, :SCH]
                        for mp in range(KO // 2):
                            wq_sb = wpool.tile([128, KO, 256], mm, tag="w")
                            nc.sync.dma_start(wq_sb,
                                              wqT_r[:, :, bass.ts(mp, 256)])
                            for mi in range(2):
                                m = 2 * mp + mi
                                ps = pp_mm.tile([128, 512], F32, tag="mm",
                                                name="ps_mm")[:, :SCH]
                                for ko in range(KO):
                                    nc.tensor.matmul(
                                        ps, wq_sb[:, ko, bass.ts(mi, 128)],
                                        hT_sb[:, ko, :],
                                        start=(ko == 0), stop=(ko == KO - 1))
                                nc.scalar.activation(
                                    qT_sb[:, m, :], ps,
                                    mybir.ActivationFunctionType.Identity,
                                    bias=bq_sb[:, m:m + 1])
                                sq = spool.tile([128, 512], mm, tag="sq",
                                                name="sq")[:, :SCH]
                                nc.vector.tensor_mul(sq, qT_sb[:, m, :],
                                                     qT_sb[:, m, :])
                                nc.tensor.matmul(ss_ps, ones_sb, sq,
                                                 start=(m == 0),
                                                 stop=(m == KO - 1))
                        # qsc = scale / rms(q)  (per s column), replicated
                        qsc = spool.tile([1, SCH], F32, name="qsc", tag="qsc")
                        nc.scalar.activation(qsc, ss_ps,
                                             mybir.ActivationFunctionType.Sqrt,
                                             scale=1.0 / D, bias=eps_sb[:1])
                        nc.vector.reciprocal(qsc, qsc)
                        nc.vector.tensor_scalar_mul(qsc, qsc, SCALE)
                        nc.gpsimd.partition_broadcast(qsc_rep, qsc)
                        # q~ = q * gq * (scale/rms): one fused DVE op per block
                        for m in range(KO):
                            nc.vector.scalar_tensor_tensor(
                                out=qT_sb[:, m, :], in0=qT_sb[:, m, :],
                                scalar=gq_sb[:, m:m + 1], in1=qsc_rep,
                                op0=mybir.AluOpType.mult,
                                op1=mybir.AluOpType.mult)

                    # ---- attention: logits transposed [L-part, s-free] ----
                    for h in range(H):
                        pt = pp_pt.tile([128, LSUB, SCH], F32, tag="pt",
                                        name="pt")
                        for lb in range(LSUB):
                            nc.tensor.matmul(
                                pt[:, lb, :], kT_sb[:, h, bass.ts(lb, 128)],
                                qT_sb[:, h, :],
                                start=(lb % LB_PER_BANK == 0),
                                stop=((lb + 1) % LB_PER_BANK == 0
                                      or lb == LSUB - 1))
                        probsT = apool.tile([128, LSUB, SCH], mm, tag="probsT")
                        nc.scalar.activation(probsT, pt,
                                             mybir.ActivationFunctionType.Exp)
                        ssum = pp_ss.tile([128, 512], F32, tag="ss",
                                          name="ssum")[:1, :SCH]
                        for lb in range(LSUB):
                            nc.tensor.matmul(ssum, ones_sb, probsT[:, lb, :],
                                             start=(lb == 0),
                                             stop=(lb == LSUB - 1))
                        rinv = spool.tile([1, SCH], F32R, tag="rinv",
                                          name="rinv")
                        with nc.allow_low_precision(
                                reason="f32r rounding of softmax 1/sum"):
                            nc.vector.reciprocal(rinv, ssum)
                        # replicate 1/sum across partitions: rank-1 f32r matmul
                        rrep_ps = pp_r.tile([128, SCH], F32, tag="rr",
                                            name="rrep_ps")
                        nc.tensor.matmul(rrep_ps, ones_row, rinv,
                                         start=True, stop=True)
                        rrep = spool.tile([128, SCH], F32, tag="rrep",
                                          name="rrep")
                        nc.scalar.activation(rrep, rrep_ps,
                                             mybir.ActivationFunctionType.Copy)
                        ops = pp_o.tile([128, SCH], F32, tag="o", name="ops")
                        for lb in range(LSUB):
                            nc.tensor.matmul(ops, v_sb[:, lb, bass.ts(h, 128)],
                                             probsT[:, lb, :],
                                             start=(lb == 0),
                                             stop=(lb == LSUB - 1))
                        o_h = opool.tile([128, SCH], mm, tag="oh", name="o_h")
                        nc.vector.tensor_mul(o_h, ops, rrep)
                        nc.sync.dma_start(oT_r[:, h, bass.ts(sc, SCH)], o_h)

        # =========== output projection ===========
        with ExitStack() as ph:
            opool = ph.enter_context(tc.tile_pool(name="oT", bufs=1))
            wpool = ph.enter_context(tc.tile_pool(name="wo", bufs=2))
            spool = ph.enter_context(tc.tile_pool(name="oscr", bufs=3))
            pp_mm = ph.enter_context(tc.tile_pool(name="ppmm", bufs=2, space="PSUM"))

            oT_all = opool.tile([128, KO, S_SHARD], mm)
            nc.sync.dma_start(oT_all, oT_r)
            bo_rep = opool.tile([128, D], mm, name="bo_rep")
            nc.gpsimd.dma_start(bo_rep, bot.ap()[None, :].to_broadcast([128, D]))
            for n in range(D // ON):
                wo_sb = wpool.tile([128, KO, ON], mm, tag="wo")
                nc.sync.dma_start(wo_sb, woT_r[:, :, bass.ts(n, ON)])
                for cs in range(S_SHARD // 128):
                    ps = pp_mm.tile([128, 512], F32, tag="mm",
                                    name="ps_mm")[:, :ON]
                    for ko in range(KO):
                        nc.tensor.matmul(ps, oT_all[:, ko, bass.ts(cs, 128)],
                                         wo_sb[:, ko, :],
                                         start=(ko == 0), stop=(ko == KO - 1))
                    o_sb = spool.tile([128, 512], F32, tag="out",
                                      name="o_sb")[:, :ON]
                    nc.vector.tensor_add(o_sb, ps, bo_rep[:, bass.ts(n, ON)])
                    nc.sync.dma_start(out_r[:, cs, bass.ts(n, ON)], o_sb)

    nc.compile()
    return nc


def _get_nc():
    global _CACHED_NC
    if _CACHED_NC is None:
        _CACHED_NC = _build()
    return _CACHED_NC


def kernel(hidden_cond, hidden_uncond, context_cond, context_uncond,
           Wq, bq, Wkv, bkv, gq, gk, Wo, bo):
    global LAST_EXEC_NS
    import ml_dtypes
    bf = ml_dtypes.bfloat16 if MM == mybir.dt.bfloat16 else np.float32
    f32 = np.float32

    nc = _get_nc()

    hid = [np.asarray(hidden_cond, f32).reshape(-1, D),
           np.asarray(hidden_uncond, f32).reshape(-1, D)]
    ctxs = [np.asarray(context_cond, f32).reshape(-1, D),
            np.asarray(context_uncond, f32).reshape(-1, D)]
    Wq = np.asarray(Wq, f32)
    Wkv = np.asarray(Wkv, f32)
    Wo = np.asarray(Wo, f32)

    common = {
        "wqT": np.ascontiguousarray(Wq.T).astype(bf),
        "wkT": np.ascontiguousarray(Wkv[:D].T).astype(bf),
        "wvT": np.ascontiguousarray(Wkv[D:].T).astype(bf),
        "woT": np.ascontiguousarray(Wo.T).astype(bf),
        "bq": np.asarray(bq, f32), "bk": np.asarray(bkv, f32)[:D],
        "bv": np.asarray(bkv, f32)[D:], "bo": np.asarray(bo, f32),
        "gq": np.asarray(gq, f32), "gk": np.asarray(gk, f32),
    }
    cTs = [np.ascontiguousarray(ctxs[g].T).astype(bf) for g in range(2)]
    in_maps = []
    for core in range(8):
        g, r = core // 4, core % 4
        hTc = np.ascontiguousarray(
            hid[g][r * S_SHARD:(r + 1) * S_SHARD].T).astype(bf)
        in_maps.append({"hT": hTc, "cT": cTs[g], **common})

    res = bass_utils.run_bass_kernel_spmd(nc, in_maps, list(range(8)),
                                          trace=TRACE)
    LAST_EXEC_NS = res.exec_time_ns

    out_c = np.concatenate([res.results[i]["out"] for i in range(4)], axis=0)
    out_u = np.concatenate([res.results[i]["out"] for i in range(4, 8)], axis=0)
    return (out_c[None], out_u[None])



# revision 6
# speedup vs baseline: 1.2645x; 1.2645x over previous
"""CFG dual cross-attention on 8 Trainium2 NeuronCores (Bass/Tile).

Sharding: the cfg axis (cond/uncond) splits the 8 cores into 2 groups of 4;
within a group the 4096 query rows are sharded 4-way (1024 rows/core) and the
K/V projection is sharded 4-way over heads.  Each core computes K^T/V for its
10 heads, the group AllGathers K/V (plus exact partial sum-of-squares rows for
the K rms-norm), and every core then runs all 40 heads of attention over its
own query rows.  The host concatenates the row shards.

Matmul operands are bf16 (fp32 PSUM accumulation); softmax/rms statistics in
fp32.  All weights are repacked host-side so every streamed weight tile is a
single fully-contiguous DMA read, and hT/cT are packed per-chunk contiguous.

Attention uses the transposed-logits formulation: logits [L-part, s-free] per
head, exp on the scalar engine, key-axis sum via ones-matmul, softmax 1/sum
via the fast approx reciprocal, replicated across partitions with a rank-1
f32r matmul issued *after* the A@V matmuls so the reciprocal latency hides
under PE work.  Per-m rms sum-of-squares matmuls are delayed by one m-tile so
the eviction->square chain never stalls the PE stream.
"""

from contextlib import ExitStack

import numpy as np

import concourse.bass as bass
import concourse.bacc as bacc
import concourse.mybir as mybir
import concourse.tile as tile
from concourse import bass_utils

EPS = 1e-6
F32 = mybir.dt.float32
F32R = mybir.dt.float32r

# ---- problem shape (nn_CFGDualCrossAttention: D=5120, H=40, S=4096, L=512) ----
D = 5120
L = 512
S_SHARD = 1024        # 4096 / 4 cores per cfg group
KO = D // 128         # contraction subtiles == heads (head_dim 128)
H = KO
LSUB = L // 128
QCH = 512             # q projection chunk (2 per shard)
SCH = 256             # attention sub-chunk (2 per q chunk)
NSUB = S_SHARD // SCH
R = 4                 # cores per cfg group
MSH = KO // R         # kv-shard m-tiles (10)
VSH = D // R          # kv-shard output cols (1280)
MM = mybir.dt.bfloat16
SCALE = float(128 ** -0.5)

# AllGather buffer layout (bf16 elements)
K_ELEMS = MSH * 128 * L           # 655360
SS_ELEMS = L                      # 512  (partial sum-of-squares row)
V_ELEMS = LSUB * 128 * VSH        # 655360
SHARD_ELEMS = K_ELEMS + SS_ELEMS + V_ELEMS

TRACE = False         # set by test harness for NTFF timing
LAST_EXEC_NS = None
_CACHED_NC = None


def _build() -> bacc.Bacc:
    mm = MM
    WKT = 5           # wk/wv stream tiles (256 cols each)
    WOT = D // 512    # wo stream tiles

    nc = bacc.Bacc("TRN2", target_bir_lowering=False, debug=False, num_devices=8)

    # ---- external inputs (host-side repacked; see kernel() below) ----
    hT_p = nc.dram_tensor("hT_p", [S_SHARD // QCH, 128, KO * QCH], mm,
                          kind="ExternalInput")
    cT_p = nc.dram_tensor("cT_p", [128, KO * L], mm, kind="ExternalInput")
    wq_p = nc.dram_tensor("wq_p", [KO, 128, KO * 128], mm, kind="ExternalInput")
    wk_p = nc.dram_tensor("wk_p", [WKT, 128, KO * 256], mm, kind="ExternalInput")
    wv_p = nc.dram_tensor("wv_p", [WKT, 128, KO * 256], mm, kind="ExternalInput")
    wo_p = nc.dram_tensor("wo_p", [WOT, 128, KO * 512], mm, kind="ExternalInput")
    gq_pm = nc.dram_tensor("gq_pm", [128, KO], F32, kind="ExternalInput")
    bqgq_pm = nc.dram_tensor("bqgq_pm", [128, KO], F32, kind="ExternalInput")
    gk_pm = nc.dram_tensor("gk_pm", [128, MSH], F32, kind="ExternalInput")
    bkgk_pm = nc.dram_tensor("bkgk_pm", [128, MSH], F32, kind="ExternalInput")
    bv_sh = nc.dram_tensor("bv_sh", [VSH], F32, kind="ExternalInput")
    bot = nc.dram_tensor("bo", [D], F32, kind="ExternalInput")
    out = nc.dram_tensor("out", [S_SHARD, D], F32, kind="ExternalOutput")

    oT_dram = nc.dram_tensor("oT_spill", [D, S_SHARD], mm)
    kv_in = nc.dram_tensor("kv_in", [SHARD_ELEMS], mm)
    # note: Shared addr_space needs >4-core groups; Local costs one extra copy
    kv_out = nc.dram_tensor("kv_out", [R * SHARD_ELEMS], mm)

    oT_r = oT_dram.rearrange("(ko p) s -> p ko s", p=128)
    out_r = out.rearrange("(cs p) n -> p cs n", p=128)

    replica_groups = [[0, 1, 2, 3], [4, 5, 6, 7]]

    with tile.TileContext(nc) as tc, ExitStack() as top:
        consts = top.enter_context(tc.tile_pool(name="consts", bufs=1))
        gq_sb = consts.tile([128, KO], F32)
        bqgq_sb = consts.tile([128, KO], F32)
        gk_sb = consts.tile([128, MSH], F32)
        bkgk_sb = consts.tile([128, MSH], F32)
        ones_sb = consts.tile([128, 1], mm)
        ones4 = consts.tile([4, 1], mm)
        ones_row = consts.tile([1, 128], F32R)
        eps_sb = consts.tile([1, 1], F32)
        nc.scalar.dma_start(gq_sb, gq_pm.ap())
        nc.scalar.dma_start(bqgq_sb, bqgq_pm.ap())
        nc.scalar.dma_start(gk_sb, gk_pm.ap())
        nc.scalar.dma_start(bkgk_sb, bkgk_pm.ap())
        ones_row_f = consts.tile([1, 128], F32)
        nc.vector.memset(ones_sb, 1.0)
        nc.vector.memset(ones4, 1.0)
        nc.vector.memset(ones_row_f, 1.0)
        with nc.allow_low_precision(reason="f32r rounding of constant ones"):
            nc.vector.tensor_copy(ones_row, ones_row_f)
        nc.vector.memset(eps_sb, EPS)

        # k^T and v (full, gathered) live across attention; freed before Oproj
        with ExitStack() as acts_scope:
            act_pool = acts_scope.enter_context(tc.tile_pool(name="acts", bufs=1))
            kT_sb = act_pool.tile([128, KO, L], mm)
            v_sb = act_pool.tile([128, LSUB, D], mm)
            kinv_rep = act_pool.tile([128, L], F32, name="kinv_rep")
            ss4_sb = act_pool.tile([4, L], mm, name="ss4")

            # =========== K + V shard (this core's 10 heads) ===========
            with ExitStack() as ph:
                cpool = ph.enter_context(tc.tile_pool(name="ctx", bufs=1))
                wpool = ph.enter_context(tc.tile_pool(name="wkv", bufs=2))
                spool = ph.enter_context(tc.tile_pool(name="kscr", bufs=2))
                pp_mm = ph.enter_context(tc.tile_pool(name="ppkv", bufs=2,
                                                      space="PSUM"))
                pp_ss = ph.enter_context(tc.tile_pool(name="ppkss", bufs=1,
                                                      space="PSUM"))

                cT_sb = cpool.tile([128, KO, L], mm)
                nc.scalar.dma_start(cT_sb, cT_p.rearrange("p (ko l) -> p ko l",
                                                          ko=KO))
                bv_rep = cpool.tile([128, VSH], mm, name="bv_rep")
                nc.gpsimd.dma_start(bv_rep,
                                    bv_sh.ap()[None, :].to_broadcast([128, VSH]))
                kTs = cpool.tile([128, MSH, L], mm, name="kTs")

                ss_ps = pp_ss.tile([128, 512], F32, name="ps_kss")
                sq_prev = None
                for t in range(WKT):
                    wk_sb = wpool.tile([128, KO, 256], mm, tag="w")
                    nc.sync.dma_start(
                        wk_sb, wk_p.ap()[t].rearrange("p (ko c) -> p ko c", ko=KO))
                    for mi in range(2):
                        m = 2 * t + mi
                        ps = pp_mm.tile([128, 512], F32, tag="mm", name="ps_k")
                        for ko in range(KO):
                            nc.tensor.matmul(ps, wk_sb[:, ko, bass.ts(mi, 128)],
                                             cT_sb[:, ko, :],
                                             start=(ko == 0), stop=(ko == KO - 1))
                        # k~ = gk*(Wk c + bk): fused scale+bias eviction
                        nc.scalar.activation(kTs[:, m, :], ps,
                                             mybir.ActivationFunctionType.Identity,
                                             bias=bkgk_sb[:, m:m + 1],
                                             scale=gk_sb[:, m:m + 1])
                        sq = spool.tile([128, 512], mm, tag="sq", name="sq")
                        nc.vector.tensor_mul(sq, kTs[:, m, :], kTs[:, m, :])
                        # delayed by one m so the evict->square chain never
                        # stalls the PE stream
                        if sq_prev is not None:
                            nc.tensor.matmul(ss_ps[:1, :L], ones_sb, sq_prev,
                                             start=(m == 1), stop=False)
                        sq_prev = sq
                nc.tensor.matmul(ss_ps[:1, :L], ones_sb, sq_prev,
                                 start=False, stop=True)
                ssk_bf = cpool.tile([1, L], mm, name="ssk_bf")
                nc.scalar.activation(ssk_bf, ss_ps[:1, :L],
                                     mybir.ActivationFunctionType.Copy)
                # spill K~^T shard + partial ss row into the AG input buffer
                nc.scalar.dma_start(
                    kv_in.ap()[:K_ELEMS].rearrange("(m p l) -> p m l",
                                                   m=MSH, p=128, l=L), kTs)
                nc.scalar.dma_start(
                    kv_in.ap()[K_ELEMS:K_ELEMS + SS_ELEMS][None, :], ssk_bf)

                # ---- V shard ----
                vs = cpool.tile([128, LSUB, VSH], mm, name="vs")
                for t in range(WKT):
                    wv_sb = wpool.tile([128, KO, 256], mm, tag="w", name="wv_sb")
                    nc.sync.dma_start(
                        wv_sb, wv_p.ap()[t].rearrange("p (ko c) -> p ko c", ko=KO))
                    for lb in range(LSUB):
                        ps = pp_mm.tile([128, 512], F32, tag="mm",
                                        name="ps_v")[:, :256]
                        for ko in range(KO):
                            nc.tensor.matmul(ps, cT_sb[:, ko, bass.ts(lb, 128)],
                                             wv_sb[:, ko, :],
                                             start=(ko == 0), stop=(ko == KO - 1))
                        nc.vector.tensor_add(vs[:, lb, bass.ts(t, 256)], ps,
                                             bv_rep[:, bass.ts(t, 256)])
                nc.scalar.dma_start(
                    kv_in.ap()[K_ELEMS + SS_ELEMS:].rearrange(
                        "(lb p n) -> p lb n", lb=LSUB, p=128, n=VSH), vs)

            # =========== AllGather K/V within each cfg group ===========
            nc.gpsimd.collective_compute(
                "AllGather", mybir.AluOpType.bypass,
                replica_groups=replica_groups,
                ins=[kv_in.ap()], outs=[kv_out.ap()])
            for r in range(R):
                base = r * SHARD_ELEMS
                nc.gpsimd.dma_start(
                    kT_sb[:, r * MSH:(r + 1) * MSH, :],
                    kv_out.ap()[base:base + K_ELEMS].rearrange(
                        "(m p l) -> p m l", m=MSH, p=128, l=L))
                nc.gpsimd.dma_start(
                    v_sb[:, :, r * VSH:(r + 1) * VSH],
                    kv_out.ap()[base + K_ELEMS + SS_ELEMS:base + SHARD_ELEMS]
                    .rearrange("(lb p n) -> p lb n", lb=LSUB, p=128, n=VSH))
            nc.gpsimd.dma_start(
                ss4_sb,
                kv_out.ap().rearrange("(r x) -> r x", r=R)[:, K_ELEMS:K_ELEMS +
                                                           SS_ELEMS])

            # =========== Q projection + attention, per q chunk ===========
            for sc in range(S_SHARD // QCH):
                with ExitStack() as ch_scope:
                    qpool = ch_scope.enter_context(tc.tile_pool(name="qch",
                                                                bufs=1))
                    spool = ch_scope.enter_context(tc.tile_pool(name="qscr",
                                                                bufs=2))
                    apool = ch_scope.enter_context(tc.tile_pool(name="attn",
                                                                bufs=2))
                    opool = ch_scope.enter_context(tc.tile_pool(name="oev",
                                                                bufs=3))

                    qT_sb = qpool.tile([128, KO, QCH], mm)
                    qsc_rep = qpool.tile([128, QCH], F32, name="qsc_rep")

                    # ---- q^T chunk = Wq @ hT (+bq), rms stats in fp32 ----
                    with ExitStack() as qproj:
                        hpool = qproj.enter_context(tc.tile_pool(name="hq",
                                                                 bufs=1))
                        wpool = qproj.enter_context(tc.tile_pool(name="wq",
                                                                 bufs=2))
                        pp_mm = qproj.enter_context(
                            tc.tile_pool(name="ppmmq", bufs=2, space="PSUM"))
                        pp_ss = qproj.enter_context(
                            tc.tile_pool(name="ppqss", bufs=1, space="PSUM"))

                        hT_sb = hpool.tile([128, KO, QCH], mm)
                        nc.scalar.dma_start(
                            hT_sb, hT_p.ap()[sc].rearrange("p (ko s) -> p ko s",
                                                           ko=KO))
                        ss_ps = pp_ss.tile([128, 512], F32,
                                           name="ps_qss")[:1, :QCH]
                        sq_prev = None
                        for m in range(KO):
                            wq_sb = wpool.tile([128, KO, 128], mm, tag="w")
                            nc.sync.dma_start(
                                wq_sb, wq_p.ap()[m].rearrange(
                                    "p (ko c) -> p ko c", ko=KO))
                            ps = pp_mm.tile([128, 512], F32, tag="mm",
                                            name="ps_q")
                            for ko in range(KO):
                                nc.tensor.matmul(ps, wq_sb[:, ko, :],
                                                 hT_sb[:, ko, :],
                                                 start=(ko == 0),
                                                 stop=(ko == KO - 1))
                            nc.scalar.activation(
                                qT_sb[:, m, :], ps,
                                mybir.ActivationFunctionType.Identity,
                                bias=bqgq_sb[:, m:m + 1],
                                scale=gq_sb[:, m:m + 1])
                            sq = spool.tile([128, 512], mm, tag="sq", name="sq")
                            nc.vector.tensor_mul(sq, qT_sb[:, m, :],
                                                 qT_sb[:, m, :])
                            if sq_prev is not None:
                                nc.tensor.matmul(ss_ps, ones_sb, sq_prev,
                                                 start=(m == 1), stop=False)
                            sq_prev = sq
                        nc.tensor.matmul(ss_ps, ones_sb, sq_prev,
                                         start=False, stop=True)
                        # qsc = scale / rms(q) per s column, replicated
                        qroot = spool.tile([1, QCH], F32, name="qroot",
                                           tag="qsc")
                        nc.scalar.activation(qroot, ss_ps,
                                             mybir.ActivationFunctionType.Sqrt,
                                             scale=1.0 / D, bias=eps_sb)
                        qsc = spool.tile([1, QCH], F32, name="qsc", tag="qsc")
                        nc.vector.reciprocal_approx_fast(qsc, qroot)
                        nc.vector.tensor_scalar_mul(qsc, qsc, SCALE)
                        nc.gpsimd.partition_broadcast(qsc_rep, qsc)
                        for m in range(KO):
                            nc.vector.tensor_mul(qT_sb[:, m, :], qT_sb[:, m, :],
                                                 qsc_rep)

                    if sc == 0:
                        # kinv from the AG'd exact partial ss rows (placed
                        # after Q0 so the PE never waits on the collective)
                        with ExitStack() as kv_scope:
                            pp_k4 = kv_scope.enter_context(
                                tc.tile_pool(name="ppk4", bufs=1, space="PSUM"))
                            kpool = kv_scope.enter_context(
                                tc.tile_pool(name="kinv", bufs=1))
                            ps4 = pp_k4.tile([128, 512], F32,
                                             name="ps4")[:1, :L]
                            nc.tensor.matmul(ps4, ones4, ss4_sb,
                                             start=True, stop=True)
                            kroot = kpool.tile([1, L], F32, name="kroot")
                            nc.scalar.activation(
                                kroot, ps4, mybir.ActivationFunctionType.Sqrt,
                                scale=1.0 / D, bias=eps_sb)
                            kinv = kpool.tile([1, L], F32, name="kinv")
                            nc.vector.reciprocal_approx_fast(kinv, kroot)
                            nc.gpsimd.partition_broadcast(kinv_rep, kinv)
                            for m in range(KO):
                                nc.vector.tensor_mul(kT_sb[:, m, :],
                                                     kT_sb[:, m, :], kinv_rep)

                    # ---- attention: logits transposed [L-part, s-free] ----
                    with ExitStack() as at_scope:
                        pp_pt = at_scope.enter_context(
                            tc.tile_pool(name="pppt", bufs=2, space="PSUM"))
                        pp_sr = at_scope.enter_context(
                            tc.tile_pool(name="ppsr", bufs=2, space="PSUM"))
                        pp_o = at_scope.enter_context(
                            tc.tile_pool(name="ppo", bufs=2, space="PSUM"))
                        for sub in range(QCH // SCH):
                            s0 = sc * (QCH // SCH) + sub
                            qsl = bass.ts(sub, SCH)
                            for h in range(H):
                                pt = pp_pt.tile([128, LSUB, SCH], F32,
                                                tag="pt", name="pt")
                                for lb in range(LSUB):
                                    nc.tensor.matmul(
                                        pt[:, lb, :],
                                        kT_sb[:, h, bass.ts(lb, 128)],
                                        qT_sb[:, h, qsl],
                                        start=(lb % 2 == 0),
                                        stop=(lb % 2 == 1))
                                probsT = apool.tile([128, LSUB, SCH], mm,
                                                    tag="probsT")
                                nc.scalar.activation(
                                    probsT, pt,
                                    mybir.ActivationFunctionType.Exp)
                                # sr bank: [:, :SCH] = rrep, [:1, SCH:2*SCH] = ssum
                                sr = pp_sr.tile([128, 512], F32, tag="sr",
                                                name="sr")
                                for lb in range(LSUB):
                                    nc.tensor.matmul(sr[:1, SCH:2 * SCH],
                                                     ones_sb, probsT[:, lb, :],
                                                     start=(lb == 0),
                                                     stop=(lb == LSUB - 1))
                                rinv = spool.tile([1, SCH], F32, tag="rinv",
                                                  name="rinv")
                                nc.vector.reciprocal_approx_fast(
                                    rinv, sr[:1, SCH:2 * SCH])
                                rinv_r = spool.tile([1, SCH], F32R,
                                                    tag="rinvr", name="rinvr")
                                with nc.allow_low_precision(
                                        reason="f32r rounding of softmax 1/sum"):
                                    nc.vector.tensor_copy(rinv_r, rinv)
                                ops = pp_o.tile([128, SCH], F32, tag="o",
                                                name="ops")
                                for lb in range(LSUB):
                                    nc.tensor.matmul(
                                        ops, v_sb[:, lb, bass.ts(h, 128)],
                                        probsT[:, lb, :],
                                        start=(lb == 0), stop=(lb == LSUB - 1))
                                # replicate 1/sum across partitions (rank-1
                                # f32r matmul) after A@V so the reciprocal
                                # latency hides under the PE stream
                                nc.tensor.matmul(sr[:, :SCH], ones_row, rinv_r,
                                                 start=True, stop=True)
                                rrep = spool.tile([128, SCH], F32, tag="rrep",
                                                  name="rrep")
                                nc.scalar.activation(
                                    rrep, sr[:, :SCH],
                                    mybir.ActivationFunctionType.Copy)
                                o_h = opool.tile([128, SCH], mm, tag="oh",
                                                 name="o_h")
                                nc.vector.tensor_mul(o_h, ops, rrep)
                                nc.sync.dma_start(
                                    oT_r[:, h, bass.ts(s0, SCH)], o_h)

        # =========== output projection ===========
        with ExitStack() as ph:
            opool = ph.enter_context(tc.tile_pool(name="oT", bufs=1))
            wpool = ph.enter_context(tc.tile_pool(name="wo", bufs=2))
            spool = ph.enter_context(tc.tile_pool(name="oscr", bufs=3))
            pp_mm = ph.enter_context(tc.tile_pool(name="ppmmo", bufs=2,
                                                  space="PSUM"))

            oT_all = opool.tile([128, KO, S_SHARD], mm)
            for c in range(NSUB):
                nc.scalar.dma_start(oT_all[:, :, bass.ts(c, SCH)],
                                    oT_r[:, :, bass.ts(c, SCH)])
            bo_rep = opool.tile([128, D], mm, name="bo_rep")
            nc.gpsimd.dma_start(bo_rep, bot.ap()[None, :].to_broadcast([128, D]))
            for t in range(D // 512):
                wo_sb = wpool.tile([128, KO, 512], mm, tag="wo")
                nc.sync.dma_start(
                    wo_sb, wo_p.ap()[t].rearrange("p (ko c) -> p ko c", ko=KO))
                for cs in range(S_SHARD // 128):
                    ps = pp_mm.tile([128, 512], F32, tag="mm", name="ps_o")
                    for ko in range(KO):
                        nc.tensor.matmul(ps, oT_all[:, ko, bass.ts(cs, 128)],
                                         wo_sb[:, ko, :],
                                         start=(ko == 0), stop=(ko == KO - 1))
                    o_sb = spool.tile([128, 512], F32, tag="out", name="o_sb")
                    nc.vector.tensor_add(o_sb, ps, bo_rep[:, bass.ts(t, 512)])
                    nc.scalar.dma_start(out_r[:, cs, bass.ts(t, 512)], o_sb)

    nc.compile()
    return nc


def _get_nc():
    global _CACHED_NC
    if _CACHED_NC is None:
        _CACHED_NC = _build()
    return _CACHED_NC


def _pack_w(wT, tc):
    """[D, N] (contraction-major transposed weight) -> [N//tc, 128, KO*tc]
    so each streamed tile is one fully-contiguous DMA read."""
    n = wT.shape[1]
    nt = n // tc
    return np.ascontiguousarray(
        wT.reshape(KO, 128, nt, tc).transpose(2, 1, 0, 3).reshape(
            nt, 128, KO * tc))


def kernel(hidden_cond, hidden_uncond, context_cond, context_uncond,
           Wq, bq, Wkv, bkv, gq, gk, Wo, bo):
    global LAST_EXEC_NS
    import ml_dtypes
    bf = ml_dtypes.bfloat16 if MM == mybir.dt.bfloat16 else np.float32
    f32 = np.float32

    nc = _get_nc()

    hid = [np.asarray(hidden_cond, f32).reshape(-1, D),
           np.asarray(hidden_uncond, f32).reshape(-1, D)]
    ctxs = [np.asarray(context_cond, f32).reshape(-1, D),
            np.asarray(context_uncond, f32).reshape(-1, D)]
    Wq = np.asarray(Wq, f32)
    Wkv = np.asarray(Wkv, f32)
    Wo = np.asarray(Wo, f32)
    bq = np.asarray(bq, f32)
    bkv = np.asarray(bkv, f32)
    bo = np.asarray(bo, f32)
    gq = np.asarray(gq, f32)
    gk = np.asarray(gk, f32)
    bk, bv = bkv[:D], bkv[D:]

    wq_pk = _pack_w(np.ascontiguousarray(Wq.T).astype(bf), 128)
    wo_pk = _pack_w(np.ascontiguousarray(Wo.T).astype(bf), 512)
    WkT = np.ascontiguousarray(Wkv[:D].T).astype(bf)
    WvT = np.ascontiguousarray(Wkv[D:].T).astype(bf)
    wk_pks = [_pack_w(WkT[:, r * VSH:(r + 1) * VSH], 256) for r in range(R)]
    wv_pks = [_pack_w(WvT[:, r * VSH:(r + 1) * VSH], 256) for r in range(R)]

    common = {
        "wq_p": wq_pk, "wo_p": wo_pk,
        "gq_pm": np.ascontiguousarray(gq.reshape(KO, 128).T),
        "bqgq_pm": np.ascontiguousarray((bq * gq).reshape(KO, 128).T),
        "bo": bo,
    }
    cT_ps = []
    for g in range(2):
        cT = np.ascontiguousarray(ctxs[g].T).astype(bf)   # [D, L]
        cT_ps.append(np.ascontiguousarray(
            cT.reshape(KO, 128, L).transpose(1, 0, 2).reshape(128, KO * L)))

    in_maps = []
    for core in range(8):
        g, r = core // 4, core % 4
        hT = np.ascontiguousarray(
            hid[g][r * S_SHARD:(r + 1) * S_SHARD].T).astype(bf)  # [D, S_SHARD]
        hT_pk = np.ascontiguousarray(
            hT.reshape(KO, 128, S_SHARD // QCH, QCH).transpose(2, 1, 0, 3)
            .reshape(S_SHARD // QCH, 128, KO * QCH))
        sl = slice(r * VSH, (r + 1) * VSH)
        in_maps.append({
            "hT_p": hT_pk, "cT_p": cT_ps[g],
            "wk_p": wk_pks[r], "wv_p": wv_pks[r],
            "gk_pm": np.ascontiguousarray(gk[sl].reshape(MSH, 128).T),
            "bkgk_pm": np.ascontiguousarray((bk * gk)[sl].reshape(MSH, 128).T),
            "bv_sh": np.ascontiguousarray(bv[sl]),
            **common,
        })

    res = bass_utils.run_bass_kernel_spmd(nc, in_maps, list(range(8)),
                                          trace=TRACE)
    LAST_EXEC_NS = res.exec_time_ns

    out_c = np.concatenate([res.results[i]["out"] for i in range(4)], axis=0)
    out_u = np.concatenate([res.results[i]["out"] for i in range(4, 8)], axis=0)
    return (out_c[None], out_u[None])


# revision 18
# speedup vs baseline: 1.2740x; 1.0075x over previous
"""CFG dual cross-attention on 8 Trainium2 NeuronCores (Bass/Tile).

Sharding: the cfg axis (cond/uncond) splits the 8 cores into 2 groups of 4;
within a group the 4096 query rows are sharded 4-way (1024 rows/core) and the
K/V projection is sharded 4-way over heads.  Each core computes K^T/V for its
10 heads, the group AllGathers K/V (plus exact partial sum-of-squares rows for
the K rms-norm), and every core then runs all 40 heads of attention over its
own query rows.  The host concatenates the row shards.

Matmul operands are bf16 (fp32 PSUM accumulation); softmax/rms statistics in
fp32.  All weights are repacked host-side so every streamed weight tile is a
single fully-contiguous DMA read, and hT/cT are packed per-chunk contiguous.

Attention uses the transposed-logits formulation: logits [L-part, s-free] per
head, exp on the scalar engine, key-axis sum via ones-matmul, softmax 1/sum
via the fast approx reciprocal, replicated across partitions with a rank-1
f32r matmul issued *after* the A@V matmuls so the reciprocal latency hides
under PE work.  Per-m rms sum-of-squares matmuls are delayed by one m-tile so
the eviction->square chain never stalls the PE stream.
"""

from contextlib import ExitStack

import numpy as np

import concourse.bass as bass
import concourse.bacc as bacc
import concourse.mybir as mybir
import concourse.tile as tile
from concourse import bass_utils

EPS = 1e-6
F32 = mybir.dt.float32
F32R = mybir.dt.float32r

# ---- problem shape (nn_CFGDualCrossAttention: D=5120, H=40, S=4096, L=512) ----
D = 5120
L = 512
S_SHARD = 1024        # 4096 / 4 cores per cfg group
KO = D // 128         # contraction subtiles == heads (head_dim 128)
H = KO
LSUB = L // 128
QCH = 512             # q projection chunk (2 per shard)
SCH = 256             # attention sub-chunk (2 per q chunk)
NSUB = S_SHARD // SCH
R = 4                 # cores per cfg group
MSH = KO // R         # kv-shard m-tiles (10)
VSH = D // R          # kv-shard output cols (1280)
MM = mybir.dt.bfloat16
SCALE = float(128 ** -0.5)

# AllGather buffer layout (bf16 elements)
K_ELEMS = MSH * 128 * L           # 655360
SS_ELEMS = L                      # 512  (partial sum-of-squares row)
V_ELEMS = LSUB * 128 * VSH        # 655360
SHARD_ELEMS = K_ELEMS + SS_ELEMS + V_ELEMS

TRACE = False         # set by test harness for NTFF timing
LAST_EXEC_NS = None
_CACHED_NC = None


def _build() -> bacc.Bacc:
    mm = MM
    WKT = 5           # wk/wv stream tiles (256 cols each)
    WOT = D // 512    # wo stream tiles

    nc = bacc.Bacc("TRN2", target_bir_lowering=False, debug=False, num_devices=8)

    # ---- external inputs (host-side repacked; see kernel() below) ----
    hT_p = nc.dram_tensor("hT_p", [S_SHARD // QCH, 128, KO * QCH], mm,
                          kind="ExternalInput")
    cT_p = nc.dram_tensor("cT_p", [128, KO * L], mm, kind="ExternalInput")
    wq_p = nc.dram_tensor("wq_p", [KO, 128, KO * 128], mm, kind="ExternalInput")
    wk_p = nc.dram_tensor("wk_p", [WKT, 128, KO * 256], mm, kind="ExternalInput")
    wv_p = nc.dram_tensor("wv_p", [WKT, 128, KO * 256], mm, kind="ExternalInput")
    wo_p = nc.dram_tensor("wo_p", [WOT, 128, KO * 512], mm, kind="ExternalInput")
    gq_pm = nc.dram_tensor("gq_pm", [128, KO], F32, kind="ExternalInput")
    bqgq_pm = nc.dram_tensor("bqgq_pm", [128, KO], F32, kind="ExternalInput")
    gk_pm = nc.dram_tensor("gk_pm", [128, MSH], F32, kind="ExternalInput")
    bkgk_pm = nc.dram_tensor("bkgk_pm", [128, MSH], F32, kind="ExternalInput")
    bv_sh = nc.dram_tensor("bv_sh", [VSH], F32, kind="ExternalInput")
    bot = nc.dram_tensor("bo", [D], F32, kind="ExternalInput")
    out = nc.dram_tensor("out", [S_SHARD, D], mm, kind="ExternalOutput")

    oT_dram = nc.dram_tensor("oT_spill", [D, S_SHARD], mm)
    kv_in = nc.dram_tensor("kv_in", [SHARD_ELEMS], mm)
    # note: Shared addr_space needs >4-core groups; Local costs one extra copy
    kv_out = nc.dram_tensor("kv_out", [R * SHARD_ELEMS], mm)

    oT_r = oT_dram.rearrange("(ko p) s -> p ko s", p=128)
    out_r = out.rearrange("(cs p) n -> p cs n", p=128)

    replica_groups = [[0, 1, 2, 3], [4, 5, 6, 7]]

    def wdma(i, dst, src):
        # alternate big streaming DMAs across the two HWDGE queues
        (nc.sync if i % 2 == 0 else nc.scalar).dma_start(dst, src)

    with tile.TileContext(nc) as tc, ExitStack() as top:
        consts = top.enter_context(tc.tile_pool(name="consts", bufs=1))
        gq_sb = consts.tile([128, KO], F32)
        bqgq_sb = consts.tile([128, KO], F32)
        gk_sb = consts.tile([128, MSH], F32)
        bkgk_sb = consts.tile([128, MSH], F32)
        ones_sb = consts.tile([128, 1], mm)
        ones4 = consts.tile([4, 1], mm)
        ones_row = consts.tile([1, 128], F32R)
        eps_sb = consts.tile([1, 1], F32)
        eps128_sb = consts.tile([1, 1], F32)
        nc.scalar.dma_start(gq_sb, gq_pm.ap())
        nc.scalar.dma_start(bqgq_sb, bqgq_pm.ap())
        nc.scalar.dma_start(gk_sb, gk_pm.ap())
        nc.scalar.dma_start(bkgk_sb, bkgk_pm.ap())
        ones_row_f = consts.tile([1, 128], F32)
        nc.vector.memset(ones_sb, 1.0)
        nc.vector.memset(ones4, 1.0)
        nc.vector.memset(ones_row_f, 1.0)
        with nc.allow_low_precision(reason="f32r rounding of constant ones"):
            nc.vector.tensor_copy(ones_row, ones_row_f)
        nc.vector.memset(eps_sb, EPS)
        nc.vector.memset(eps128_sb, 128.0 * EPS)

        # k^T and v (full, gathered) live across attention; freed before Oproj
        with ExitStack() as acts_scope:
            act_pool = acts_scope.enter_context(tc.tile_pool(name="acts", bufs=1))
            kT_sb = act_pool.tile([128, KO, L], mm)
            v_sb = act_pool.tile([128, LSUB, D], mm)
            kinv_rep = act_pool.tile([128, L], F32, name="kinv_rep")
            ss4_sb = act_pool.tile([4, L], mm, name="ss4")

            # =========== K + V shard (this core's 10 heads) ===========
            with ExitStack() as ph:
                cpool = ph.enter_context(tc.tile_pool(name="ctx", bufs=1))
                wpool = ph.enter_context(tc.tile_pool(name="wkv", bufs=2))
                spool = ph.enter_context(tc.tile_pool(name="kscr", bufs=2))
                pp_mm = ph.enter_context(tc.tile_pool(name="ppkv", bufs=2,
                                                      space="PSUM"))
                pp_ss = ph.enter_context(tc.tile_pool(name="ppkss", bufs=1,
                                                      space="PSUM"))

                cT_sb = cpool.tile([128, KO, L], mm)
                cT_r = cT_p.rearrange("p (ko l) -> p ko l", ko=KO)
                for q in range(4):
                    wdma(q, cT_sb[:, bass.ts(q, 10), :], cT_r[:, bass.ts(q, 10), :])
                bv_rep = cpool.tile([128, VSH], mm, name="bv_rep")
                nc.gpsimd.dma_start(bv_rep,
                                    bv_sh.ap()[None, :].to_broadcast([128, VSH]))
                kTs = cpool.tile([128, MSH, L], mm, name="kTs")

                ss_ps = pp_ss.tile([128, 512], F32, name="ps_kss")
                sq_prev = None
                for t in range(WKT):
                    wk_sb = wpool.tile([128, KO, 256], mm, tag="w")
                    wdma(t, wk_sb,
                         wk_p.ap()[t].rearrange("p (ko c) -> p ko c", ko=KO))
                    for mi in range(2):
                        m = 2 * t + mi
                        ps = pp_mm.tile([128, 512], F32, tag="mm", name="ps_k")
                        for ko in range(KO):
                            nc.tensor.matmul(ps, wk_sb[:, ko, bass.ts(mi, 128)],
                                             cT_sb[:, ko, :],
                                             start=(ko == 0), stop=(ko == KO - 1))
                        # k~ = gk*(Wk c + bk): fused scale+bias eviction
                        nc.scalar.activation(kTs[:, m, :], ps,
                                             mybir.ActivationFunctionType.Identity,
                                             bias=bkgk_sb[:, m:m + 1],
                                             scale=gk_sb[:, m:m + 1])
                        sq = spool.tile([128, 512], mm, tag="sq", name="sq")
                        nc.vector.tensor_mul(sq, kTs[:, m, :], kTs[:, m, :])
                        # delayed by one m so the evict->square chain never
                        # stalls the PE stream
                        if sq_prev is not None:
                            nc.tensor.matmul(ss_ps[:1, :L], ones_sb, sq_prev,
                                             start=(m == 1), stop=False)
                        sq_prev = sq
                nc.tensor.matmul(ss_ps[:1, :L], ones_sb, sq_prev,
                                 start=False, stop=True)
                ssk_bf = cpool.tile([1, L], mm, name="ssk_bf")
                nc.scalar.activation(ssk_bf, ss_ps[:1, :L],
                                     mybir.ActivationFunctionType.Copy)
                # spill K~^T shard + partial ss row into the AG input buffer
                nc.scalar.dma_start(
                    kv_in.ap()[:K_ELEMS].rearrange("(m p l) -> p m l",
                                                   m=MSH, p=128, l=L), kTs)
                nc.scalar.dma_start(
                    kv_in.ap()[K_ELEMS:K_ELEMS + SS_ELEMS][None, :], ssk_bf)

                # ---- V shard ----
                vs = cpool.tile([128, LSUB, VSH], mm, name="vs")
                for t in range(WKT):
                    wv_sb = wpool.tile([128, KO, 256], mm, tag="w", name="wv_sb")
                    wdma(t + 1, wv_sb,
                         wv_p.ap()[t].rearrange("p (ko c) -> p ko c", ko=KO))
                    for lb in range(LSUB):
                        ps = pp_mm.tile([128, 512], F32, tag="mm",
                                        name="ps_v")[:, :256]
                        for ko in range(KO):
                            nc.tensor.matmul(ps, cT_sb[:, ko, bass.ts(lb, 128)],
                                             wv_sb[:, ko, :],
                                             start=(ko == 0), stop=(ko == KO - 1))
                        nc.vector.tensor_add(vs[:, lb, bass.ts(t, 256)], ps,
                                             bv_rep[:, bass.ts(t, 256)])
                nc.scalar.dma_start(
                    kv_in.ap()[K_ELEMS + SS_ELEMS:].rearrange(
                        "(lb p n) -> p lb n", lb=LSUB, p=128, n=VSH), vs)

            # =========== AllGather K/V within each cfg group ===========
            nc.gpsimd.collective_compute(
                "AllGather", mybir.AluOpType.bypass,
                replica_groups=replica_groups,
                ins=[kv_in.ap()], outs=[kv_out.ap()])
            for r in range(R):
                base = r * SHARD_ELEMS
                nc.gpsimd.dma_start(
                    kT_sb[:, r * MSH:(r + 1) * MSH, :],
                    kv_out.ap()[base:base + K_ELEMS].rearrange(
                        "(m p l) -> p m l", m=MSH, p=128, l=L))
                nc.gpsimd.dma_start(
                    v_sb[:, :, r * VSH:(r + 1) * VSH],
                    kv_out.ap()[base + K_ELEMS + SS_ELEMS:base + SHARD_ELEMS]
                    .rearrange("(lb p n) -> p lb n", lb=LSUB, p=128, n=VSH))
            nc.gpsimd.dma_start(
                ss4_sb,
                kv_out.ap().rearrange("(r x) -> r x", r=R)[:, K_ELEMS:K_ELEMS +
                                                           SS_ELEMS])

            # =========== Q projection + attention, per q chunk ===========
            for sc in range(S_SHARD // QCH):
                with ExitStack() as ch_scope:
                    qpool = ch_scope.enter_context(tc.tile_pool(name="qch",
                                                                bufs=1))
                    spool = ch_scope.enter_context(tc.tile_pool(name="qscr",
                                                                bufs=2))
                    apool = ch_scope.enter_context(tc.tile_pool(name="attn",
                                                                bufs=2))
                    opool = ch_scope.enter_context(tc.tile_pool(name="oev",
                                                                bufs=2))

                    qT_sb = qpool.tile([128, KO, QCH], mm)
                    qsc_rep = qpool.tile([128, QCH], F32, name="qsc_rep")

                    # ---- q^T chunk = Wq @ hT (+bq), rms stats in fp32 ----
                    with ExitStack() as qproj:
                        hpool = qproj.enter_context(tc.tile_pool(name="hq",
                                                                 bufs=1))
                        wpool = qproj.enter_context(tc.tile_pool(name="wq",
                                                                 bufs=2))
                        pp_mm = qproj.enter_context(
                            tc.tile_pool(name="ppmmq", bufs=2, space="PSUM"))
                        pp_ss = qproj.enter_context(
                            tc.tile_pool(name="ppqss", bufs=1, space="PSUM"))

                        hT_sb = hpool.tile([128, KO, QCH], mm)
                        hT_r = hT_p.ap()[sc].rearrange("p (ko s) -> p ko s",
                                                       ko=KO)
                        for q in range(4):
                            wdma(q, hT_sb[:, bass.ts(q, 10), :],
                                 hT_r[:, bass.ts(q, 10), :])
                        ss_ps = pp_ss.tile([128, 512], F32,
                                           name="ps_qss")[:1, :QCH]
                        sq_prev = None
                        for m in range(KO):
                            wq_sb = wpool.tile([128, KO, 128], mm, tag="w")
                            wdma(m, wq_sb, wq_p.ap()[m].rearrange(
                                "p (ko c) -> p ko c", ko=KO))
                            ps = pp_mm.tile([128, 512], F32, tag="mm",
                                            name="ps_q")
                            for ko in range(KO):
                                nc.tensor.matmul(ps, wq_sb[:, ko, :],
                                                 hT_sb[:, ko, :],
                                                 start=(ko == 0),
                                                 stop=(ko == KO - 1))
                            nc.scalar.activation(
                                qT_sb[:, m, :], ps,
                                mybir.ActivationFunctionType.Identity,
                                bias=bqgq_sb[:, m:m + 1],
                                scale=gq_sb[:, m:m + 1])
                            sq = spool.tile([128, 512], mm, tag="sq", name="sq")
                            nc.vector.tensor_mul(sq, qT_sb[:, m, :],
                                                 qT_sb[:, m, :])
                            if sq_prev is not None:
                                nc.tensor.matmul(ss_ps, ones_sb, sq_prev,
                                                 start=(m == 1), stop=False)
                            sq_prev = sq
                            if sc == 0 and m == 25:
                                # kinv from the AG'd exact partial ss rows —
                                # mid-Q0 so the collective is long done and
                                # the tiny PE op never stalls the stream
                                ps4 = pp_mm.tile([128, 512], F32, tag="mm",
                                                 name="ps4")[:1, :L]
                                nc.tensor.matmul(ps4, ones4, ss4_sb,
                                                 start=True, stop=True)
                                kroot = act_pool.tile([1, L], F32,
                                                      name="kroot")
                                nc.scalar.activation(
                                    kroot, ps4,
                                    mybir.ActivationFunctionType.Sqrt,
                                    scale=1.0 / D, bias=eps_sb)
                                kinv = act_pool.tile([1, L], F32, name="kinv")
                                nc.vector.reciprocal_approx_fast(kinv, kroot)
                                nc.gpsimd.partition_broadcast(kinv_rep, kinv)
                                for g in range(KO // 8):
                                    nc.vector.tensor_mul(
                                        kT_sb[:, bass.ts(g, 8), :],
                                        kT_sb[:, bass.ts(g, 8), :],
                                        kinv_rep[:, None, :].to_broadcast(
                                            [128, 8, L]))
                        nc.tensor.matmul(ss_ps, ones_sb, sq_prev,
                                         start=False, stop=True)
                        # qsc = scale / rms(q) per s column (scale folded
                        # into the sqrt), replicated across partitions
                        qroot = spool.tile([1, QCH], F32, name="qroot",
                                           tag="qsc")
                        nc.scalar.activation(qroot, ss_ps,
                                             mybir.ActivationFunctionType.Sqrt,
                                             scale=128.0 / D, bias=eps128_sb)
                        qsc = spool.tile([1, QCH], F32, name="qsc", tag="qsc")
                        nc.vector.reciprocal_approx_fast(qsc, qroot)
                        nc.gpsimd.partition_broadcast(qsc_rep, qsc)
                        for g in range(KO // 8):
                            nc.vector.tensor_mul(
                                qT_sb[:, bass.ts(g, 8), :],
                                qT_sb[:, bass.ts(g, 8), :],
                                qsc_rep[:, None, :].to_broadcast(
                                    [128, 8, QCH]))

                    # ---- attention: logits transposed [L-part, s-free] ----
                    with ExitStack() as at_scope:
                        pp_pt = at_scope.enter_context(
                            tc.tile_pool(name="pppt", bufs=2, space="PSUM"))
                        pp_sr = at_scope.enter_context(
                            tc.tile_pool(name="ppsr", bufs=2, space="PSUM"))
                        pp_o = at_scope.enter_context(
                            tc.tile_pool(name="ppo", bufs=2, space="PSUM"))
                        for sub in range(QCH // SCH):
                            s0 = sc * (QCH // SCH) + sub
                            qsl = bass.ts(sub, SCH)
                            for h in range(H):
                                pt = pp_pt.tile([128, LSUB, SCH], F32,
                                                tag="pt", name="pt")
                                for lb in range(LSUB):
                                    nc.tensor.matmul(
                                        pt[:, lb, :],
                                        kT_sb[:, h, bass.ts(lb, 128)],
                                        qT_sb[:, h, qsl],
                                        start=(lb % 2 == 0),
                                        stop=(lb % 2 == 1))
                                probsT = apool.tile([128, LSUB, SCH], mm,
                                                    tag="probsT")
                                nc.scalar.activation(
                                    probsT, pt,
                                    mybir.ActivationFunctionType.Exp)
                                # sr bank: [:, :SCH] = rrep, [:1, SCH:2*SCH] = ssum
                                sr = pp_sr.tile([128, 512], F32, tag="sr",
                                                name="sr")
                                for lb in range(LSUB):
                                    nc.tensor.matmul(sr[:1, SCH:2 * SCH],
                                                     ones_sb, probsT[:, lb, :],
                                                     start=(lb == 0),
                                                     stop=(lb == LSUB - 1))
                                rinv = spool.tile([1, SCH], F32, tag="rinv",
                                                  name="rinv")
                                nc.vector.reciprocal_approx_fast(
                                    rinv, sr[:1, SCH:2 * SCH])
                                rinv_r = spool.tile([1, SCH], F32R,
                                                    tag="rinvr", name="rinvr")
                                with nc.allow_low_precision(
                                        reason="f32r rounding of softmax 1/sum"):
                                    nc.vector.tensor_copy(rinv_r, rinv)
                                ops = pp_o.tile([128, SCH], F32, tag="o",
                                                name="ops")
                                for lb in range(LSUB):
                                    nc.tensor.matmul(
                                        ops, v_sb[:, lb, bass.ts(h, 128)],
                                        probsT[:, lb, :],
                                        start=(lb == 0), stop=(lb == LSUB - 1))
                                # replicate 1/sum across partitions (rank-1
                                # f32r matmul) after A@V so the reciprocal
                                # latency hides under the PE stream
                                nc.tensor.matmul(sr[:, :SCH], ones_row, rinv_r,
                                                 start=True, stop=True)
                                rrep = spool.tile([128, SCH], F32, tag="rrep",
                                                  name="rrep")
                                nc.scalar.activation(
                                    rrep, sr[:, :SCH],
                                    mybir.ActivationFunctionType.Copy)
                                o_h = opool.tile([128, SCH], mm, tag="oh",
                                                 name="o_h")
                                nc.vector.tensor_mul(o_h, ops, rrep)
                                nc.sync.dma_start(
                                    oT_r[:, h, bass.ts(s0, SCH)], o_h)

        # =========== output projection ===========
        with ExitStack() as ph:
            opool = ph.enter_context(tc.tile_pool(name="oT", bufs=1))
            wpool = ph.enter_context(tc.tile_pool(name="wo", bufs=2))
            spool = ph.enter_context(tc.tile_pool(name="oscr", bufs=3))
            pp_mm = ph.enter_context(tc.tile_pool(name="ppmmo", bufs=2,
                                                  space="PSUM"))

            oT_all = opool.tile([128, KO, S_SHARD], mm)
            bo_rep = opool.tile([128, D], mm, name="bo_rep")
            nc.gpsimd.dma_start(bo_rep, bot.ap()[None, :].to_broadcast([128, D]))
            # first wo tiles prefetch during the attention tail via SWDGE
            # (no deps), before the spill-gated oT readbacks enter the queue
            wo_tiles = []
            for t in range(2):
                wo_sb = wpool.tile([128, KO, 512], mm, tag="wo", name="wo_sb")
                nc.gpsimd.dma_start(
                    wo_sb, wo_p.ap()[t].rearrange("p (ko c) -> p ko c", ko=KO))
                wo_tiles.append(wo_sb)
            for c in range(NSUB):
                # gpsimd SWDGE: does not contend with attn spills (sync) or
                # the hwdge weight streams
                nc.gpsimd.dma_start(oT_all[:, :, bass.ts(c, SCH)],
                                    oT_r[:, :, bass.ts(c, SCH)])
            for t in range(D // 512):
                if t < 2:
                    wo_sb = wo_tiles[t]
                else:
                    wo_sb = wpool.tile([128, KO, 512], mm, tag="wo",
                                       name="wo_sb")
                    wdma(t, wo_sb,
                         wo_p.ap()[t].rearrange("p (ko c) -> p ko c", ko=KO))
                for cs in range(S_SHARD // 128):
                    ps = pp_mm.tile([128, 512], F32, tag="mm", name="ps_o")
                    for ko in range(KO):
                        nc.tensor.matmul(ps, oT_all[:, ko, bass.ts(cs, 128)],
                                         wo_sb[:, ko, :],
                                         start=(ko == 0), stop=(ko == KO - 1))
                    o_sb = spool.tile([128, 512], mm, tag="out", name="o_sb")
                    nc.vector.tensor_add(o_sb, ps, bo_rep[:, bass.ts(t, 512)])
                    nc.scalar.dma_start(out_r[:, cs, bass.ts(t, 512)], o_sb)

    nc.compile()
    return nc


def _get_nc():
    global _CACHED_NC
    if _CACHED_NC is None:
        _CACHED_NC = _build()
    return _CACHED_NC


def _pack_w(wT, tc):
    """[D, N] (contraction-major transposed weight) -> [N//tc, 128, KO*tc]
    so each streamed tile is one fully-contiguous DMA read."""
    n = wT.shape[1]
    nt = n // tc
    return np.ascontiguousarray(
        wT.reshape(KO, 128, nt, tc).transpose(2, 1, 0, 3).reshape(
            nt, 128, KO * tc))


def kernel(hidden_cond, hidden_uncond, context_cond, context_uncond,
           Wq, bq, Wkv, bkv, gq, gk, Wo, bo):
    global LAST_EXEC_NS
    import ml_dtypes
    bf = ml_dtypes.bfloat16 if MM == mybir.dt.bfloat16 else np.float32
    f32 = np.float32

    nc = _get_nc()

    hid = [np.asarray(hidden_cond, f32).reshape(-1, D),
           np.asarray(hidden_uncond, f32).reshape(-1, D)]
    ctxs = [np.asarray(context_cond, f32).reshape(-1, D),
            np.asarray(context_uncond, f32).reshape(-1, D)]
    Wq = np.asarray(Wq, f32)
    Wkv = np.asarray(Wkv, f32)
    Wo = np.asarray(Wo, f32)
    bq = np.asarray(bq, f32)
    bkv = np.asarray(bkv, f32)
    bo = np.asarray(bo, f32)
    gq = np.asarray(gq, f32)
    gk = np.asarray(gk, f32)
    bk, bv = bkv[:D], bkv[D:]

    wq_pk = _pack_w(np.ascontiguousarray(Wq.T).astype(bf), 128)
    wo_pk = _pack_w(np.ascontiguousarray(Wo.T).astype(bf), 512)
    WkT = np.ascontiguousarray(Wkv[:D].T).astype(bf)
    WvT = np.ascontiguousarray(Wkv[D:].T).astype(bf)
    wk_pks = [_pack_w(WkT[:, r * VSH:(r + 1) * VSH], 256) for r in range(R)]
    wv_pks = [_pack_w(WvT[:, r * VSH:(r + 1) * VSH], 256) for r in range(R)]

    common = {
        "wq_p": wq_pk, "wo_p": wo_pk,
        "gq_pm": np.ascontiguousarray(gq.reshape(KO, 128).T),
        "bqgq_pm": np.ascontiguousarray((bq * gq).reshape(KO, 128).T),
        "bo": bo,
    }
    cT_ps = []
    for g in range(2):
        cT = np.ascontiguousarray(ctxs[g].T).astype(bf)   # [D, L]
        cT_ps.append(np.ascontiguousarray(
            cT.reshape(KO, 128, L).transpose(1, 0, 2).reshape(128, KO * L)))

    in_maps = []
    for core in range(8):
        g, r = core // 4, core % 4
        hT = np.ascontiguousarray(
            hid[g][r * S_SHARD:(r + 1) * S_SHARD].T).astype(bf)  # [D, S_SHARD]
        hT_pk = np.ascontiguousarray(
            hT.reshape(KO, 128, S_SHARD // QCH, QCH).transpose(2, 1, 0, 3)
            .reshape(S_SHARD // QCH, 128, KO * QCH))
        sl = slice(r * VSH, (r + 1) * VSH)
        in_maps.append({
            "hT_p": hT_pk, "cT_p": cT_ps[g],
            "wk_p": wk_pks[r], "wv_p": wv_pks[r],
            "gk_pm": np.ascontiguousarray(gk[sl].reshape(MSH, 128).T),
            "bkgk_pm": np.ascontiguousarray((bk * gk)[sl].reshape(MSH, 128).T),
            "bv_sh": np.ascontiguousarray(bv[sl]),
            **common,
        })

    res = bass_utils.run_bass_kernel_spmd(nc, in_maps, list(range(8)),
                                          trace=TRACE)
    LAST_EXEC_NS = res.exec_time_ns

    out_c = np.concatenate(
        [np.asarray(res.results[i]["out"], f32) for i in range(4)], axis=0)
    out_u = np.concatenate(
        [np.asarray(res.results[i]["out"], f32) for i in range(4, 8)], axis=0)
    return (out_c[None], out_u[None])


# revision 32
# speedup vs baseline: 1.2926x; 1.0146x over previous
"""CFG dual cross-attention on 8 Trainium2 NeuronCores (Bass/Tile).

Sharding: the cfg axis (cond/uncond) splits the 8 cores into 2 groups of 4;
within a group the 4096 query rows are sharded 4-way (1024 rows/core) and the
K/V projection is sharded 4-way over heads.  Each core computes K^T/V for its
10 heads, the group AllGathers K/V (plus exact partial sum-of-squares rows for
the K rms-norm), and every core then runs all 40 heads of attention over its
own query rows.  The host concatenates the row shards.

Matmul operands are bf16 (fp32 PSUM accumulation); softmax/rms statistics in
fp32.  All weights are repacked host-side so every streamed weight tile is a
single fully-contiguous DMA read, and hT/cT are packed per-chunk contiguous.

Attention uses the transposed-logits formulation: logits [L-part, s-free] per
head, exp on the scalar engine, key-axis sum via ones-matmul, softmax 1/sum
via the fast approx reciprocal, replicated across partitions with a rank-1
f32r matmul issued *after* the A@V matmuls so the reciprocal latency hides
under PE work.  Per-m rms sum-of-squares matmuls are delayed by one m-tile so
the eviction->square chain never stalls the PE stream.
"""

from contextlib import ExitStack

import numpy as np

import concourse.bass as bass
import concourse.bacc as bacc
import concourse.mybir as mybir
import concourse.tile as tile
from concourse import bass_utils

EPS = 1e-6
F32 = mybir.dt.float32
F32R = mybir.dt.float32r

# ---- problem shape (nn_CFGDualCrossAttention: D=5120, H=40, S=4096, L=512) ----
D = 5120
L = 512
S_SHARD = 1024        # 4096 / 4 cores per cfg group
KO = D // 128         # contraction subtiles == heads (head_dim 128)
H = KO
LSUB = L // 128
QCH = 512             # q projection chunk (2 per shard)
SCH = 256             # attention sub-chunk (2 per q chunk)
NSUB = S_SHARD // SCH
R = 4                 # cores per cfg group
MSH = KO // R         # kv-shard m-tiles (10)
VSH = D // R          # kv-shard output cols (1280)
MM = mybir.dt.bfloat16
SCALE = float(128 ** -0.5)

# AllGather buffer layout (bf16 elements)
K_ELEMS = MSH * 128 * L           # 655360
SS_ELEMS = L                      # 512  (partial sum-of-squares row)
V_ELEMS = LSUB * 128 * VSH        # 655360
SHARD_ELEMS = K_ELEMS + SS_ELEMS + V_ELEMS

TRACE = False         # set by test harness for NTFF timing
LAST_EXEC_NS = None
_CACHED_NC = None


def _build() -> bacc.Bacc:
    mm = MM
    WKT = 5           # wk/wv stream tiles (256 cols each)
    WOT = D // 512    # wo stream tiles

    nc = bacc.Bacc("TRN2", target_bir_lowering=False, debug=False, num_devices=8)

    # ---- external inputs (host-side repacked; see kernel() below) ----
    hT_p = nc.dram_tensor("hT_p", [S_SHARD // QCH, 128, KO * QCH], mm,
                          kind="ExternalInput")
    cT_p = nc.dram_tensor("cT_p", [128, KO * L], mm, kind="ExternalInput")
    wq_p = nc.dram_tensor("wq_p", [KO, 128, KO * 128], mm, kind="ExternalInput")
    wk_p = nc.dram_tensor("wk_p", [WKT, 128, KO * 256], mm, kind="ExternalInput")
    wv_p = nc.dram_tensor("wv_p", [WKT, 128, KO * 256], mm, kind="ExternalInput")
    wo_p = nc.dram_tensor("wo_p", [WOT, 128, KO * 512], mm, kind="ExternalInput")
    gq_pm = nc.dram_tensor("gq_pm", [128, KO], F32, kind="ExternalInput")
    bqgq_pm = nc.dram_tensor("bqgq_pm", [128, KO], F32, kind="ExternalInput")
    gk_pm = nc.dram_tensor("gk_pm", [128, MSH], F32, kind="ExternalInput")
    bkgk_pm = nc.dram_tensor("bkgk_pm", [128, MSH], F32, kind="ExternalInput")
    bv_sh = nc.dram_tensor("bv_sh", [VSH], F32, kind="ExternalInput")
    bot = nc.dram_tensor("bo", [D], F32, kind="ExternalInput")
    out = nc.dram_tensor("out", [S_SHARD, D], mm, kind="ExternalOutput")

    oT_dram = nc.dram_tensor("oT_spill", [D, S_SHARD], mm)
    kv_in = nc.dram_tensor("kv_in", [SHARD_ELEMS], mm)
    # note: Shared addr_space needs >4-core groups; Local costs one extra copy
    kv_out = nc.dram_tensor("kv_out", [R * SHARD_ELEMS], mm)

    oT_r = oT_dram.rearrange("(ko p) s -> p ko s", p=128)
    out_r = out.rearrange("(cs p) n -> p cs n", p=128)

    replica_groups = [[0, 1, 2, 3], [4, 5, 6, 7]]

    def wdma(i, dst, src):
        # alternate big streaming DMAs across the two HWDGE queues
        (nc.sync if i % 2 == 0 else nc.scalar).dma_start(dst, src)

    def wdma2(dst, src):
        # split one weight tile across both HWDGE queues (halved latency)
        half = dst.shape[1] // 2
        nc.sync.dma_start(dst[:, :half], src[:, :half])
        nc.scalar.dma_start(dst[:, half:], src[:, half:])

    with tile.TileContext(nc) as tc, ExitStack() as top:
        consts = top.enter_context(tc.tile_pool(name="consts", bufs=1))
        gq_sb = consts.tile([128, KO], F32)
        bqgq_sb = consts.tile([128, KO], F32)
        gk_sb = consts.tile([128, MSH], F32)
        bkgk_sb = consts.tile([128, MSH], F32)
        ones_sb = consts.tile([128, 1], mm)
        ones4 = consts.tile([4, 1], mm)
        eps_sb = consts.tile([1, 1], F32)
        eps128_sb = consts.tile([1, 1], F32)
        nc.scalar.dma_start(gq_sb, gq_pm.ap())
        nc.scalar.dma_start(bqgq_sb, bqgq_pm.ap())
        nc.scalar.dma_start(gk_sb, gk_pm.ap())
        nc.scalar.dma_start(bkgk_sb, bkgk_pm.ap())
        nc.vector.memset(ones_sb, 1.0)
        nc.vector.memset(ones4, 1.0)
        nc.vector.memset(eps_sb, EPS)
        nc.vector.memset(eps128_sb, 128.0 * EPS)

        # k^T and v (full, gathered) live across attention; freed before Oproj
        with ExitStack() as acts_scope:
            act_pool = acts_scope.enter_context(tc.tile_pool(name="acts", bufs=1))
            kT_sb = act_pool.tile([128, KO, L], mm)
            v_sb = act_pool.tile([128, LSUB, D], mm)
            kinv_rep = act_pool.tile([128, L], F32, name="kinv_rep")
            ss4_sb = act_pool.tile([4, L], mm, name="ss4")

            # =========== K + V shard (this core's 10 heads) ===========
            with ExitStack() as ph:
                cpool = ph.enter_context(tc.tile_pool(name="ctx", bufs=1))
                wpool = ph.enter_context(tc.tile_pool(name="wkv", bufs=2))
                spool = ph.enter_context(tc.tile_pool(name="kscr", bufs=2))
                pp_mm = ph.enter_context(tc.tile_pool(name="ppkv", bufs=2,
                                                      space="PSUM"))
                pp_ss = ph.enter_context(tc.tile_pool(name="ppkss", bufs=1,
                                                      space="PSUM"))

                cT_sb = cpool.tile([128, KO, L], mm)
                cT_r = cT_p.rearrange("p (ko l) -> p ko l", ko=KO)
                # interleave first wk tiles with the cT quarters so the K
                # matmuls (per-ko granular) start as early as possible
                wdma(0, cT_sb[:, bass.ts(0, 10), :], cT_r[:, bass.ts(0, 10), :])
                wdma(1, cT_sb[:, bass.ts(1, 10), :], cT_r[:, bass.ts(1, 10), :])
                wk_tiles = []
                for t in range(2):
                    wk_sb = wpool.tile([128, KO, 256], mm, tag="w", name="wk_sb")
                    wdma2(wk_sb,
                          wk_p.ap()[t].rearrange("p (ko c) -> p ko c", ko=KO))
                    wk_tiles.append(wk_sb)
                wdma(0, cT_sb[:, bass.ts(2, 10), :], cT_r[:, bass.ts(2, 10), :])
                wdma(1, cT_sb[:, bass.ts(3, 10), :], cT_r[:, bass.ts(3, 10), :])
                bv_rep = cpool.tile([128, VSH], mm, name="bv_rep")
                nc.gpsimd.dma_start(bv_rep,
                                    bv_sh.ap()[None, :].to_broadcast([128, VSH]))
                kTs = cpool.tile([128, MSH, L], mm, name="kTs")

                ss_ps = pp_ss.tile([128, 512], F32, name="ps_kss")
                sq_prev = None
                for t in range(WKT):
                    if t < 2:
                        wk_sb = wk_tiles[t]
                    else:
                        wk_sb = wpool.tile([128, KO, 256], mm, tag="w",
                                           name="wk_sb")
                        wdma2(wk_sb,
                              wk_p.ap()[t].rearrange("p (ko c) -> p ko c",
                                                     ko=KO))
                    for mi in range(2):
                        m = 2 * t + mi
                        ps = pp_mm.tile([128, 512], F32, tag="mm", name="ps_k")
                        for ko in range(KO):
                            nc.tensor.matmul(ps, wk_sb[:, ko, bass.ts(mi, 128)],
                                             cT_sb[:, ko, :],
                                             start=(ko == 0), stop=(ko == KO - 1))
                        # k~ = gk*(Wk c + bk): fused scale+bias eviction
                        nc.scalar.activation(kTs[:, m, :], ps,
                                             mybir.ActivationFunctionType.Identity,
                                             bias=bkgk_sb[:, m:m + 1],
                                             scale=gk_sb[:, m:m + 1])
                        sq = spool.tile([128, 512], mm, tag="sq", name="sq")
                        nc.vector.tensor_mul(sq, kTs[:, m, :], kTs[:, m, :])
                        # delayed by one m so the evict->square chain never
                        # stalls the PE stream
                        if sq_prev is not None:
                            nc.tensor.matmul(ss_ps[:1, :L], ones_sb, sq_prev,
                                             start=(m == 1), stop=False)
                        sq_prev = sq
                nc.tensor.matmul(ss_ps[:1, :L], ones_sb, sq_prev,
                                 start=False, stop=True)
                ssk_bf = cpool.tile([1, L], mm, name="ssk_bf")
                nc.scalar.activation(ssk_bf, ss_ps[:1, :L],
                                     mybir.ActivationFunctionType.Copy)
                # spill K~^T shard + partial ss row into the AG input buffer
                nc.scalar.dma_start(
                    kv_in.ap()[:K_ELEMS].rearrange("(m p l) -> p m l",
                                                   m=MSH, p=128, l=L), kTs)
                nc.scalar.dma_start(
                    kv_in.ap()[K_ELEMS:K_ELEMS + SS_ELEMS][None, :], ssk_bf)

                # ---- V shard ----
                vs = cpool.tile([128, LSUB, VSH], mm, name="vs")
                for t in range(WKT):
                    wv_sb = wpool.tile([128, KO, 256], mm, tag="w", name="wv_sb")
                    wdma2(wv_sb,
                          wv_p.ap()[t].rearrange("p (ko c) -> p ko c", ko=KO))
                    for lb in range(LSUB):
                        ps = pp_mm.tile([128, 512], F32, tag="mm",
                                        name="ps_v")[:, :256]
                        for ko in range(KO):
                            nc.tensor.matmul(ps, cT_sb[:, ko, bass.ts(lb, 128)],
                                             wv_sb[:, ko, :],
                                             start=(ko == 0), stop=(ko == KO - 1))
                        nc.vector.tensor_add(vs[:, lb, bass.ts(t, 256)], ps,
                                             bv_rep[:, bass.ts(t, 256)])
                nc.scalar.dma_start(
                    kv_in.ap()[K_ELEMS + SS_ELEMS:].rearrange(
                        "(lb p n) -> p lb n", lb=LSUB, p=128, n=VSH), vs)

            # =========== AllGather K/V within each cfg group ===========
            nc.gpsimd.collective_compute(
                "AllGather", mybir.AluOpType.bypass,
                replica_groups=replica_groups,
                ins=[kv_in.ap()], outs=[kv_out.ap()])
            for r in range(R):
                base = r * SHARD_ELEMS
                nc.gpsimd.dma_start(
                    kT_sb[:, r * MSH:(r + 1) * MSH, :],
                    kv_out.ap()[base:base + K_ELEMS].rearrange(
                        "(m p l) -> p m l", m=MSH, p=128, l=L))
                nc.gpsimd.dma_start(
                    v_sb[:, :, r * VSH:(r + 1) * VSH],
                    kv_out.ap()[base + K_ELEMS + SS_ELEMS:base + SHARD_ELEMS]
                    .rearrange("(lb p n) -> p lb n", lb=LSUB, p=128, n=VSH))
            nc.gpsimd.dma_start(
                ss4_sb,
                kv_out.ap().rearrange("(r x) -> r x", r=R)[:, K_ELEMS:K_ELEMS +
                                                           SS_ELEMS])

            # =========== Q projection + attention, per q chunk ===========
            for sc in range(S_SHARD // QCH):
                with ExitStack() as ch_scope:
                    qpool = ch_scope.enter_context(tc.tile_pool(name="qch",
                                                                bufs=1))
                    spool = ch_scope.enter_context(tc.tile_pool(name="qscr",
                                                                bufs=2))
                    apool = ch_scope.enter_context(tc.tile_pool(name="attn",
                                                                bufs=2))
                    opool = ch_scope.enter_context(tc.tile_pool(name="oev",
                                                                bufs=2))

                    qT_sb = qpool.tile([128, KO, QCH], mm)
                    qsc_rep = qpool.tile([128, QCH], F32, name="qsc_rep")

                    # ---- q^T chunk = Wq @ hT (+bq), rms stats in fp32 ----
                    with ExitStack() as qproj:
                        hpool = qproj.enter_context(tc.tile_pool(name="hq",
                                                                 bufs=1))
                        wpool = qproj.enter_context(tc.tile_pool(name="wq",
                                                                 bufs=2))
                        pp_mm = qproj.enter_context(
                            tc.tile_pool(name="ppmmq", bufs=2, space="PSUM"))
                        pp_ss = qproj.enter_context(
                            tc.tile_pool(name="ppqss", bufs=1, space="PSUM"))

                        hT_sb = hpool.tile([128, KO, QCH], mm)
                        hT_r = hT_p.ap()[sc].rearrange("p (ko s) -> p ko s",
                                                       ko=KO)
                        # emission order = queue order: first wq tiles slot
                        # between the hT quarters on each queue, and all hT
                        # writes are emitted before any matmul reads them
                        wdma(0, hT_sb[:, bass.ts(0, 10), :],
                             hT_r[:, bass.ts(0, 10), :])
                        wdma(1, hT_sb[:, bass.ts(1, 10), :],
                             hT_r[:, bass.ts(1, 10), :])
                        wq_tiles = []
                        for m in range(2):
                            wq_sb = wpool.tile([128, KO, 128], mm, tag="w",
                                               name="wq_sb")
                            wdma(m, wq_sb, wq_p.ap()[m].rearrange(
                                "p (ko c) -> p ko c", ko=KO))
                            wq_tiles.append(wq_sb)
                        wdma(0, hT_sb[:, bass.ts(2, 10), :],
                             hT_r[:, bass.ts(2, 10), :])
                        wdma(1, hT_sb[:, bass.ts(3, 10), :],
                             hT_r[:, bass.ts(3, 10), :])
                        ss_ps = pp_ss.tile([128, 512], F32,
                                           name="ps_qss")[:1, :QCH]
                        sq_prev = None
                        for m in range(KO):
                            if m < 2:
                                wq_sb = wq_tiles[m]
                            else:
                                wq_sb = wpool.tile([128, KO, 128], mm, tag="w",
                                                   name="wq_sb")
                                wdma(m, wq_sb, wq_p.ap()[m].rearrange(
                                    "p (ko c) -> p ko c", ko=KO))
                            ps = pp_mm.tile([128, 512], F32, tag="mm",
                                            name="ps_q")
                            for ko in range(KO):
                                nc.tensor.matmul(ps, wq_sb[:, ko, :],
                                                 hT_sb[:, ko, :],
                                                 start=(ko == 0),
                                                 stop=(ko == KO - 1))
                            nc.scalar.activation(
                                qT_sb[:, m, :], ps,
                                mybir.ActivationFunctionType.Identity,
                                bias=bqgq_sb[:, m:m + 1],
                                scale=gq_sb[:, m:m + 1])
                            sq = spool.tile([128, 512], mm, tag="sq", name="sq")
                            nc.vector.tensor_mul(sq, qT_sb[:, m, :],
                                                 qT_sb[:, m, :])
                            if sq_prev is not None:
                                nc.tensor.matmul(ss_ps, ones_sb, sq_prev,
                                                 start=(m == 1), stop=False)
                            sq_prev = sq
                            if sc == 0 and m == 25:
                                # kinv from the AG'd exact partial ss rows —
                                # mid-Q0 so the collective is long done and
                                # the tiny PE op never stalls the stream
                                ps4 = pp_mm.tile([128, 512], F32, tag="mm",
                                                 name="ps4")[:1, :L]
                                nc.tensor.matmul(ps4, ones4, ss4_sb,
                                                 start=True, stop=True)
                                kroot = act_pool.tile([1, L], F32,
                                                      name="kroot")
                                nc.scalar.activation(
                                    kroot, ps4,
                                    mybir.ActivationFunctionType.Sqrt,
                                    scale=1.0 / D, bias=eps_sb)
                                kinv = act_pool.tile([1, L], F32, name="kinv")
                                nc.vector.reciprocal_approx_fast(kinv, kroot)
                                nc.gpsimd.partition_broadcast(kinv_rep, kinv)
                                for g in range(KO // 8):
                                    nc.vector.tensor_mul(
                                        kT_sb[:, bass.ts(g, 8), :],
                                        kT_sb[:, bass.ts(g, 8), :],
                                        kinv_rep[:, None, :].to_broadcast(
                                            [128, 8, L]))
                        nc.tensor.matmul(ss_ps, ones_sb, sq_prev,
                                         start=False, stop=True)
                        # qsc = scale / rms(q) per s column (scale folded
                        # into the sqrt), replicated across partitions
                        qroot = spool.tile([1, QCH], F32, name="qroot",
                                           tag="qsc")
                        nc.scalar.activation(qroot, ss_ps,
                                             mybir.ActivationFunctionType.Sqrt,
                                             scale=128.0 / D, bias=eps128_sb)
                        qsc = spool.tile([1, QCH], F32, name="qsc", tag="qsc")
                        nc.vector.reciprocal_approx_fast(qsc, qroot)
                        nc.gpsimd.partition_broadcast(qsc_rep, qsc)
                        for g in range(KO // 8):
                            nc.vector.tensor_mul(
                                qT_sb[:, bass.ts(g, 8), :],
                                qT_sb[:, bass.ts(g, 8), :],
                                qsc_rep[:, None, :].to_broadcast(
                                    [128, 8, QCH]))

                    # ---- attention: logits transposed [L-part, s-free] ----
                    with ExitStack() as at_scope:
                        pp_pt = at_scope.enter_context(
                            tc.tile_pool(name="pppt", bufs=2, space="PSUM"))
                        pp_sr = at_scope.enter_context(
                            tc.tile_pool(name="ppsr", bufs=2, space="PSUM"))
                        pp_o = at_scope.enter_context(
                            tc.tile_pool(name="ppo", bufs=2, space="PSUM"))
                        for sub in range(QCH // SCH):
                            s0 = sc * (QCH // SCH) + sub
                            qsl = bass.ts(sub, SCH)
                            for h in range(H):
                                pt = pp_pt.tile([128, LSUB, SCH], F32,
                                                tag="pt", name="pt")
                                for lb in range(LSUB):
                                    nc.tensor.matmul(
                                        pt[:, lb, :],
                                        kT_sb[:, h, bass.ts(lb, 128)],
                                        qT_sb[:, h, qsl],
                                        start=(lb % 2 == 0),
                                        stop=(lb % 2 == 1))
                                probsT = apool.tile([128, LSUB, SCH], mm,
                                                    tag="probsT")
                                nc.scalar.activation(
                                    probsT, pt,
                                    mybir.ActivationFunctionType.Exp)
                                sr = pp_sr.tile([128, 512], F32, tag="sr",
                                                name="sr")
                                for lb in range(LSUB):
                                    nc.tensor.matmul(sr[:1, :SCH],
                                                     ones_sb, probsT[:, lb, :],
                                                     start=(lb == 0),
                                                     stop=(lb == LSUB - 1))
                                rinv = spool.tile([1, SCH], F32, tag="rinv",
                                                  name="rinv")
                                nc.vector.reciprocal_approx_fast(
                                    rinv, sr[:1, :SCH])
                                ops = pp_o.tile([128, SCH], F32, tag="o",
                                                name="ops")
                                for lb in range(LSUB):
                                    nc.tensor.matmul(
                                        ops, v_sb[:, lb, bass.ts(h, 128)],
                                        probsT[:, lb, :],
                                        start=(lb == 0), stop=(lb == LSUB - 1))
                                # replicate 1/sum across partitions off the
                                # PE stream (gpsimd is otherwise idle here)
                                rrep = spool.tile([128, SCH], F32, tag="rrep",
                                                  name="rrep")
                                nc.gpsimd.partition_broadcast(rrep, rinv)
                                o_h = opool.tile([128, SCH], mm, tag="oh",
                                                 name="o_h")
                                nc.vector.tensor_mul(o_h, ops, rrep)
                                nc.sync.dma_start(
                                    oT_r[:, h, bass.ts(s0, SCH)], o_h)

        # =========== output projection ===========
        with ExitStack() as ph:
            opool = ph.enter_context(tc.tile_pool(name="oT", bufs=1))
            wpool = ph.enter_context(tc.tile_pool(name="wo", bufs=2))
            spool = ph.enter_context(tc.tile_pool(name="oscr", bufs=3))
            pp_mm = ph.enter_context(tc.tile_pool(name="ppmmo", bufs=2,
                                                  space="PSUM"))

            oT_all = opool.tile([128, KO, S_SHARD], mm)
            bo_rep = opool.tile([128, D], mm, name="bo_rep")
            nc.gpsimd.dma_start(bo_rep, bot.ap()[None, :].to_broadcast([128, D]))
            # first wo tiles prefetch during the attention tail via SWDGE
            # (no deps), before the spill-gated oT readbacks enter the queue
            wo_tiles = []
            for t in range(2):
                wo_sb = wpool.tile([128, KO, 512], mm, tag="wo", name="wo_sb")
                nc.gpsimd.dma_start(
                    wo_sb, wo_p.ap()[t].rearrange("p (ko c) -> p ko c", ko=KO))
                wo_tiles.append(wo_sb)
            for c in range(NSUB):
                # scalar HWDGE is idle during attention; the last sub-chunk
                # (gated on the final spills) is split across both queues
                if c < NSUB - 1:
                    nc.scalar.dma_start(oT_all[:, :, bass.ts(c, SCH)],
                                        oT_r[:, :, bass.ts(c, SCH)])
                else:
                    csl = bass.ts(c, SCH)
                    nc.sync.dma_start(oT_all[:, :20, csl], oT_r[:, :20, csl])
                    nc.scalar.dma_start(oT_all[:, 20:, csl], oT_r[:, 20:, csl])
            for t in range(D // 512):
                if t < 2:
                    wo_sb = wo_tiles[t]
                else:
                    wo_sb = wpool.tile([128, KO, 512], mm, tag="wo",
                                       name="wo_sb")
                    wdma(t, wo_sb,
                         wo_p.ap()[t].rearrange("p (ko c) -> p ko c", ko=KO))
                for cs in range(S_SHARD // 128):
                    ps = pp_mm.tile([128, 512], F32, tag="mm", name="ps_o")
                    for ko in range(KO):
                        nc.tensor.matmul(ps, oT_all[:, ko, bass.ts(cs, 128)],
                                         wo_sb[:, ko, :],
                                         start=(ko == 0), stop=(ko == KO - 1))
                    o_sb = spool.tile([128, 512], mm, tag="out", name="o_sb")
                    nc.vector.tensor_add(o_sb, ps, bo_rep[:, bass.ts(t, 512)])
                    nc.scalar.dma_start(out_r[:, cs, bass.ts(t, 512)], o_sb)

    nc.compile()
    return nc


def _get_nc():
    global _CACHED_NC
    if _CACHED_NC is None:
        _CACHED_NC = _build()
    return _CACHED_NC


def _pack_w(wT, tc):
    """[D, N] (contraction-major transposed weight) -> [N//tc, 128, KO*tc]
    so each streamed tile is one fully-contiguous DMA read."""
    n = wT.shape[1]
    nt = n // tc
    return np.ascontiguousarray(
        wT.reshape(KO, 128, nt, tc).transpose(2, 1, 0, 3).reshape(
            nt, 128, KO * tc))


def kernel(hidden_cond, hidden_uncond, context_cond, context_uncond,
           Wq, bq, Wkv, bkv, gq, gk, Wo, bo):
    global LAST_EXEC_NS
    import ml_dtypes
    bf = ml_dtypes.bfloat16 if MM == mybir.dt.bfloat16 else np.float32
    f32 = np.float32

    nc = _get_nc()

    hid = [np.asarray(hidden_cond, f32).reshape(-1, D),
           np.asarray(hidden_uncond, f32).reshape(-1, D)]
    ctxs = [np.asarray(context_cond, f32).reshape(-1, D),
            np.asarray(context_uncond, f32).reshape(-1, D)]
    Wq = np.asarray(Wq, f32)
    Wkv = np.asarray(Wkv, f32)
    Wo = np.asarray(Wo, f32)
    bq = np.asarray(bq, f32)
    bkv = np.asarray(bkv, f32)
    bo = np.asarray(bo, f32)
    gq = np.asarray(gq, f32)
    gk = np.asarray(gk, f32)
    bk, bv = bkv[:D], bkv[D:]

    wq_pk = _pack_w(np.ascontiguousarray(Wq.T).astype(bf), 128)
    wo_pk = _pack_w(np.ascontiguousarray(Wo.T).astype(bf), 512)
    WkT = np.ascontiguousarray(Wkv[:D].T).astype(bf)
    WvT = np.ascontiguousarray(Wkv[D:].T).astype(bf)
    wk_pks = [_pack_w(WkT[:, r * VSH:(r + 1) * VSH], 256) for r in range(R)]
    wv_pks = [_pack_w(WvT[:, r * VSH:(r + 1) * VSH], 256) for r in range(R)]

    common = {
        "wq_p": wq_pk, "wo_p": wo_pk,
        "gq_pm": np.ascontiguousarray(gq.reshape(KO, 128).T),
        "bqgq_pm": np.ascontiguousarray((bq * gq).reshape(KO, 128).T),
        "bo": bo,
    }
    cT_ps = []
    for g in range(2):
        cT = np.ascontiguousarray(ctxs[g].T).astype(bf)   # [D, L]
        cT_ps.append(np.ascontiguousarray(
            cT.reshape(KO, 128, L).transpose(1, 0, 2).reshape(128, KO * L)))

    in_maps = []
    for core in range(8):
        g, r = core // 4, core % 4
        hT = np.ascontiguousarray(
            hid[g][r * S_SHARD:(r + 1) * S_SHARD].T).astype(bf)  # [D, S_SHARD]
        hT_pk = np.ascontiguousarray(
            hT.reshape(KO, 128, S_SHARD // QCH, QCH).transpose(2, 1, 0, 3)
            .reshape(S_SHARD // QCH, 128, KO * QCH))
        sl = slice(r * VSH, (r + 1) * VSH)
        in_maps.append({
            "hT_p": hT_pk, "cT_p": cT_ps[g],
            "wk_p": wk_pks[r], "wv_p": wv_pks[r],
            "gk_pm": np.ascontiguousarray(gk[sl].reshape(MSH, 128).T),
            "bkgk_pm": np.ascontiguousarray((bk * gk)[sl].reshape(MSH, 128).T),
            "bv_sh": np.ascontiguousarray(bv[sl]),
            **common,
        })

    res = bass_utils.run_bass_kernel_spmd(nc, in_maps, list(range(8)),
                                          trace=TRACE)
    LAST_EXEC_NS = res.exec_time_ns

    out_c = np.concatenate(
        [np.asarray(res.results[i]["out"], f32) for i in range(4)], axis=0)
    out_u = np.concatenate(
        [np.asarray(res.results[i]["out"], f32) for i in range(4, 8)], axis=0)
    return (out_c[None], out_u[None])


# revision 37
# speedup vs baseline: 1.3156x; 1.0178x over previous
"""CFG dual cross-attention on 8 Trainium2 NeuronCores (Bass/Tile).

Sharding: the cfg axis (cond/uncond) splits the 8 cores into 2 groups of 4;
within a group the 4096 query rows are sharded 4-way (1024 rows/core) and the
K/V projection is sharded 4-way over heads.  Each core computes K^T/V for its
10 heads, the group AllGathers K/V (plus exact partial sum-of-squares rows for
the K rms-norm), and every core then runs all 40 heads of attention over its
own query rows.  The host concatenates the row shards.

Matmul operands are bf16 (fp32 PSUM accumulation); softmax/rms statistics in
fp32.  All weights are repacked host-side so every streamed weight tile is a
single fully-contiguous DMA read, and hT/cT are packed per-chunk contiguous.

Attention uses the transposed-logits formulation: logits [L-part, s-free] per
head, exp on the scalar engine, key-axis sum via ones-matmul, softmax 1/sum
via the fast approx reciprocal, replicated across partitions with a rank-1
f32r matmul issued *after* the A@V matmuls so the reciprocal latency hides
under PE work.  Per-m rms sum-of-squares matmuls are delayed by one m-tile so
the eviction->square chain never stalls the PE stream.
"""

from contextlib import ExitStack

import numpy as np

import concourse.bass as bass
import concourse.bacc as bacc
import concourse.mybir as mybir
import concourse.tile as tile
from concourse import bass_utils

EPS = 1e-6
F32 = mybir.dt.float32
F32R = mybir.dt.float32r

# ---- problem shape (nn_CFGDualCrossAttention: D=5120, H=40, S=4096, L=512) ----
D = 5120
L = 512
S_SHARD = 1024        # 4096 / 4 cores per cfg group
KO = D // 128         # contraction subtiles == heads (head_dim 128)
H = KO
LSUB = L // 128
QCH = 512             # q projection chunk (2 per shard)
SCH = 256             # attention sub-chunk (2 per q chunk)
NSUB = S_SHARD // SCH
R = 4                 # cores per cfg group
MSH = KO // R         # kv-shard m-tiles (10)
VSH = D // R          # kv-shard output cols (1280)
MM = mybir.dt.bfloat16
SCALE = float(128 ** -0.5)

# AllGather buffer layout (bf16 elements)
K_ELEMS = MSH * 128 * L           # 655360
SS_ELEMS = L                      # 512  (partial sum-of-squares row)
V_ELEMS = LSUB * 128 * VSH        # 655360
SHARD_ELEMS = K_ELEMS + SS_ELEMS + V_ELEMS

TRACE = False         # set by test harness for NTFF timing
LAST_EXEC_NS = None
_CACHED_NC = None


def _build() -> bacc.Bacc:
    mm = MM
    WKT = 5           # wk/wv stream tiles (256 cols each)
    WOT = D // 512    # wo stream tiles

    nc = bacc.Bacc("TRN2", target_bir_lowering=False, debug=False, num_devices=8)

    # ---- external inputs (host-side repacked; see kernel() below) ----
    hT_p = nc.dram_tensor("hT_p", [128, KO * S_SHARD], mm,
                          kind="ExternalInput")
    cT_p = nc.dram_tensor("cT_p", [128, KO * L], mm, kind="ExternalInput")
    wq_p = nc.dram_tensor("wq_p", [KO, 128, KO * 128], mm, kind="ExternalInput")
    wk_p = nc.dram_tensor("wk_p", [WKT, 128, KO * 256], mm, kind="ExternalInput")
    wv_p = nc.dram_tensor("wv_p", [WKT, 128, KO * 256], mm, kind="ExternalInput")
    wo_p = nc.dram_tensor("wo_p", [WOT, 128, KO * 512], mm, kind="ExternalInput")
    gq_pm = nc.dram_tensor("gq_pm", [128, KO], F32, kind="ExternalInput")
    bqgq_pm = nc.dram_tensor("bqgq_pm", [128, KO], F32, kind="ExternalInput")
    gk_pm = nc.dram_tensor("gk_pm", [128, MSH], F32, kind="ExternalInput")
    bkgk_pm = nc.dram_tensor("bkgk_pm", [128, MSH], F32, kind="ExternalInput")
    bv_sh = nc.dram_tensor("bv_sh", [VSH], F32, kind="ExternalInput")
    bot = nc.dram_tensor("bo", [D], F32, kind="ExternalInput")
    out = nc.dram_tensor("out", [S_SHARD, D], mm, kind="ExternalOutput")

    oT_dram = nc.dram_tensor("oT_spill", [D, S_SHARD], mm)
    qT_dram = nc.dram_tensor("qT_spill", [KO, 128, S_SHARD], mm)
    kv_in = nc.dram_tensor("kv_in", [SHARD_ELEMS], mm)
    # note: Shared addr_space needs >4-core groups; Local costs one extra copy
    kv_out = nc.dram_tensor("kv_out", [R * SHARD_ELEMS], mm)

    oT_r = oT_dram.rearrange("(ko p) s -> p ko s", p=128)
    out_r = out.rearrange("(cs p) n -> p cs n", p=128)

    replica_groups = [[0, 1, 2, 3], [4, 5, 6, 7]]

    def wdma(i, dst, src):
        # alternate big streaming DMAs across the two HWDGE queues
        (nc.sync if i % 2 == 0 else nc.scalar).dma_start(dst, src)

    def wdma2(dst, src):
        # split one weight tile across both HWDGE queues (halved latency)
        half = dst.shape[1] // 2
        nc.sync.dma_start(dst[:, :half], src[:, :half])
        nc.scalar.dma_start(dst[:, half:], src[:, half:])

    with tile.TileContext(nc) as tc, ExitStack() as top:
        consts = top.enter_context(tc.tile_pool(name="consts", bufs=1))
        gq_sb = consts.tile([128, KO], F32)
        bqgq_sb = consts.tile([128, KO], F32)
        gk_sb = consts.tile([128, MSH], F32)
        bkgk_sb = consts.tile([128, MSH], F32)
        ones_sb = consts.tile([128, 1], mm)
        ones4 = consts.tile([4, 1], mm)
        eps_sb = consts.tile([1, 1], F32)
        eps128_sb = consts.tile([1, 1], F32)
        nc.scalar.dma_start(gq_sb, gq_pm.ap())
        nc.scalar.dma_start(bqgq_sb, bqgq_pm.ap())
        nc.scalar.dma_start(gk_sb, gk_pm.ap())
        nc.scalar.dma_start(bkgk_sb, bkgk_pm.ap())
        nc.vector.memset(ones_sb, 1.0)
        nc.vector.memset(ones4, 1.0)
        nc.vector.memset(eps_sb, EPS)
        nc.vector.memset(eps128_sb, 128.0 * EPS)

        # k^T and v (full, gathered) live across attention; freed before Oproj
        with ExitStack() as acts_scope:
            act_pool = acts_scope.enter_context(tc.tile_pool(name="acts", bufs=1))
            kT_sb = act_pool.tile([128, KO, L], mm)
            v_sb = act_pool.tile([128, LSUB, D], mm)
            kinv_rep = act_pool.tile([128, L], F32, name="kinv_rep")
            ss4_sb = act_pool.tile([4, L], mm, name="ss4")

            # =========== K + V shard (this core's 10 heads) ===========
            with ExitStack() as ph:
                cpool = ph.enter_context(tc.tile_pool(name="ctx", bufs=1))
                wpool = ph.enter_context(tc.tile_pool(name="wkv", bufs=2))
                spool = ph.enter_context(tc.tile_pool(name="kscr", bufs=2))
                pp_mm = ph.enter_context(tc.tile_pool(name="ppkv", bufs=2,
                                                      space="PSUM"))
                pp_ss = ph.enter_context(tc.tile_pool(name="ppkss", bufs=1,
                                                      space="PSUM"))

                cT_sb = cpool.tile([128, KO, L], mm)
                cT_r = cT_p.rearrange("p (ko l) -> p ko l", ko=KO)
                # interleave first wk tiles with the cT quarters so the K
                # matmuls (per-ko granular) start as early as possible
                wdma(0, cT_sb[:, bass.ts(0, 10), :], cT_r[:, bass.ts(0, 10), :])
                wdma(1, cT_sb[:, bass.ts(1, 10), :], cT_r[:, bass.ts(1, 10), :])
                wk_tiles = []
                for t in range(2):
                    wk_sb = wpool.tile([128, KO, 256], mm, tag="w", name="wk_sb")
                    wdma2(wk_sb,
                          wk_p.ap()[t].rearrange("p (ko c) -> p ko c", ko=KO))
                    wk_tiles.append(wk_sb)
                wdma(0, cT_sb[:, bass.ts(2, 10), :], cT_r[:, bass.ts(2, 10), :])
                wdma(1, cT_sb[:, bass.ts(3, 10), :], cT_r[:, bass.ts(3, 10), :])
                bv_rep = cpool.tile([128, VSH], mm, name="bv_rep")
                nc.gpsimd.dma_start(bv_rep,
                                    bv_sh.ap()[None, :].to_broadcast([128, VSH]))
                kTs = cpool.tile([128, MSH, L], mm, name="kTs")

                ss_ps = pp_ss.tile([128, 512], F32, name="ps_kss")
                sq_prev = None
                for t in range(WKT):
                    if t < 2:
                        wk_sb = wk_tiles[t]
                    else:
                        wk_sb = wpool.tile([128, KO, 256], mm, tag="w",
                                           name="wk_sb")
                        wdma2(wk_sb,
                              wk_p.ap()[t].rearrange("p (ko c) -> p ko c",
                                                     ko=KO))
                    for mi in range(2):
                        m = 2 * t + mi
                        ps = pp_mm.tile([128, 512], F32, tag="mm", name="ps_k")
                        for ko in range(KO):
                            nc.tensor.matmul(ps, wk_sb[:, ko, bass.ts(mi, 128)],
                                             cT_sb[:, ko, :],
                                             start=(ko == 0), stop=(ko == KO - 1))
                        # k~ = gk*(Wk c + bk): fused scale+bias eviction
                        nc.scalar.activation(kTs[:, m, :], ps,
                                             mybir.ActivationFunctionType.Identity,
                                             bias=bkgk_sb[:, m:m + 1],
                                             scale=gk_sb[:, m:m + 1])
                        sq = spool.tile([128, 512], mm, tag="sq", name="sq")
                        nc.vector.tensor_mul(sq, kTs[:, m, :], kTs[:, m, :])
                        # delayed by one m so the evict->square chain never
                        # stalls the PE stream
                        if sq_prev is not None:
                            nc.tensor.matmul(ss_ps[:1, :L], ones_sb, sq_prev,
                                             start=(m == 1), stop=False)
                        sq_prev = sq
                nc.tensor.matmul(ss_ps[:1, :L], ones_sb, sq_prev,
                                 start=False, stop=True)
                ssk_bf = cpool.tile([1, L], mm, name="ssk_bf")
                nc.scalar.activation(ssk_bf, ss_ps[:1, :L],
                                     mybir.ActivationFunctionType.Copy)
                # spill K~^T shard + partial ss row into the AG input buffer
                nc.scalar.dma_start(
                    kv_in.ap()[:K_ELEMS].rearrange("(m p l) -> p m l",
                                                   m=MSH, p=128, l=L), kTs)
                nc.scalar.dma_start(
                    kv_in.ap()[K_ELEMS:K_ELEMS + SS_ELEMS][None, :], ssk_bf)

                # ---- V shard ----
                vs = cpool.tile([128, LSUB, VSH], mm, name="vs")
                for t in range(WKT):
                    wv_sb = wpool.tile([128, KO, 256], mm, tag="w", name="wv_sb")
                    wdma2(wv_sb,
                          wv_p.ap()[t].rearrange("p (ko c) -> p ko c", ko=KO))
                    for lb in range(LSUB):
                        ps = pp_mm.tile([128, 512], F32, tag="mm",
                                        name="ps_v")[:, :256]
                        for ko in range(KO):
                            nc.tensor.matmul(ps, cT_sb[:, ko, bass.ts(lb, 128)],
                                             wv_sb[:, ko, :],
                                             start=(ko == 0), stop=(ko == KO - 1))
                        nc.vector.tensor_add(vs[:, lb, bass.ts(t, 256)], ps,
                                             bv_rep[:, bass.ts(t, 256)])
                nc.scalar.dma_start(
                    kv_in.ap()[K_ELEMS + SS_ELEMS:].rearrange(
                        "(lb p n) -> p lb n", lb=LSUB, p=128, n=VSH), vs)

            # =========== AllGather K/V within each cfg group ===========
            nc.gpsimd.collective_compute(
                "AllGather", mybir.AluOpType.bypass,
                replica_groups=replica_groups,
                ins=[kv_in.ap()], outs=[kv_out.ap()])
            for r in range(R):
                base = r * SHARD_ELEMS
                nc.gpsimd.dma_start(
                    kT_sb[:, r * MSH:(r + 1) * MSH, :],
                    kv_out.ap()[base:base + K_ELEMS].rearrange(
                        "(m p l) -> p m l", m=MSH, p=128, l=L))
                nc.gpsimd.dma_start(
                    v_sb[:, :, r * VSH:(r + 1) * VSH],
                    kv_out.ap()[base + K_ELEMS + SS_ELEMS:base + SHARD_ELEMS]
                    .rearrange("(lb p n) -> p lb n", lb=LSUB, p=128, n=VSH))
            nc.gpsimd.dma_start(
                ss4_sb,
                kv_out.ap().rearrange("(r x) -> r x", r=R)[:, K_ELEMS:K_ELEMS +
                                                           SS_ELEMS])

            # ===== Q projection: one pass over Wq, q^T spilled to DRAM =====
            qsc_a = act_pool.tile([1, QCH], F32, name="qsc_a")
            qsc_b = act_pool.tile([1, QCH], F32, name="qsc_b")
            qsc_h = [qsc_a, qsc_b]
            with ExitStack() as qproj:
                hpool = qproj.enter_context(tc.tile_pool(name="hq", bufs=1))
                wpool = qproj.enter_context(tc.tile_pool(name="wq", bufs=2))
                qmpool = qproj.enter_context(tc.tile_pool(name="qtm", bufs=3))
                spool = qproj.enter_context(tc.tile_pool(name="qscr", bufs=2))
                pp_mm = qproj.enter_context(
                    tc.tile_pool(name="ppmmq", bufs=2, space="PSUM"))
                pp_ss = qproj.enter_context(
                    tc.tile_pool(name="ppqss", bufs=1, space="PSUM"))

                hT_sb = hpool.tile([128, KO, S_SHARD], mm)
                hT_r = hT_p.rearrange("p (ko s) -> p ko s", ko=KO)
                # emission order = queue order: first wq tiles slot between
                # the hT quarters on each queue, and all hT writes are
                # emitted before any matmul reads them
                wdma(0, hT_sb[:, bass.ts(0, 10), :], hT_r[:, bass.ts(0, 10), :])
                wdma(1, hT_sb[:, bass.ts(1, 10), :], hT_r[:, bass.ts(1, 10), :])
                wq_tiles = []
                for m in range(2):
                    wq_sb = wpool.tile([128, KO, 128], mm, tag="w",
                                       name="wq_sb")
                    wdma(m, wq_sb, wq_p.ap()[m].rearrange(
                        "p (ko c) -> p ko c", ko=KO))
                    wq_tiles.append(wq_sb)
                wdma(0, hT_sb[:, bass.ts(2, 10), :], hT_r[:, bass.ts(2, 10), :])
                wdma(1, hT_sb[:, bass.ts(3, 10), :], hT_r[:, bass.ts(3, 10), :])
                ss_a = pp_ss.tile([128, 512], F32, name="ps_qss_a")
                ss_b = pp_ss.tile([128, 512], F32, name="ps_qss_b")
                ss_h = [ss_a, ss_b]
                sq_prev = [None, None]
                for m in range(KO):
                    if m < 2:
                        wq_sb = wq_tiles[m]
                    else:
                        wq_sb = wpool.tile([128, KO, 128], mm, tag="w",
                                           name="wq_sb")
                        wdma(m, wq_sb, wq_p.ap()[m].rearrange(
                            "p (ko c) -> p ko c", ko=KO))
                    qTm = qmpool.tile([128, 2, QCH], mm, tag="qtm", name="qTm")
                    for hf in range(2):
                        ps = pp_mm.tile([128, 512], F32, tag="mm", name="ps_q")
                        for ko in range(KO):
                            nc.tensor.matmul(
                                ps, wq_sb[:, ko, :],
                                hT_sb[:, ko, bass.ts(hf, QCH)],
                                start=(ko == 0), stop=(ko == KO - 1))
                        nc.scalar.activation(
                            qTm[:, hf, :], ps,
                            mybir.ActivationFunctionType.Identity,
                            bias=bqgq_sb[:, m:m + 1], scale=gq_sb[:, m:m + 1])
                        sq = spool.tile([128, 512], mm, tag=f"sq{hf}",
                                        name="sq")
                        nc.vector.tensor_mul(sq, qTm[:, hf, :], qTm[:, hf, :])
                        if sq_prev[hf] is not None:
                            nc.tensor.matmul(ss_h[hf][:1, :QCH], ones_sb,
                                             sq_prev[hf],
                                             start=(m == 1), stop=False)
                        sq_prev[hf] = sq
                    wdma(m, qT_dram.ap()[m], qTm)
                    if m == 25:
                        # kinv from the AG'd exact partial ss rows — mid-Q
                        # so the collective is long done and the tiny PE op
                        # never stalls the stream
                        ps4 = pp_mm.tile([128, 512], F32, tag="mm",
                                         name="ps4")[:1, :L]
                        nc.tensor.matmul(ps4, ones4, ss4_sb,
                                         start=True, stop=True)
                        kroot = act_pool.tile([1, L], F32, name="kroot")
                        nc.scalar.activation(
                            kroot, ps4, mybir.ActivationFunctionType.Sqrt,
                            scale=1.0 / D, bias=eps_sb)
                        kinv = act_pool.tile([1, L], F32, name="kinv")
                        nc.vector.reciprocal_approx_fast(kinv, kroot)
                        nc.gpsimd.partition_broadcast(kinv_rep, kinv)
                        for g in range(KO // 8):
                            nc.vector.tensor_mul(
                                kT_sb[:, bass.ts(g, 8), :],
                                kT_sb[:, bass.ts(g, 8), :],
                                kinv_rep[:, None, :].to_broadcast([128, 8, L]))
                for hf in range(2):
                    nc.tensor.matmul(ss_h[hf][:1, :QCH], ones_sb, sq_prev[hf],
                                     start=False, stop=True)
                    # qsc = scale / rms(q) per s column (scale folded into
                    # the sqrt)
                    qroot = spool.tile([1, QCH], F32, name="qroot", tag="qsc")
                    nc.scalar.activation(qroot, ss_h[hf][:1, :QCH],
                                         mybir.ActivationFunctionType.Sqrt,
                                         scale=128.0 / D, bias=eps128_sb)
                    nc.vector.reciprocal_approx_fast(qsc_h[hf], qroot)

            # ========== attention: logits transposed [L-part, s-free] ======
            with ExitStack() as at_scope:
                qcpool = at_scope.enter_context(tc.tile_pool(name="qtc",
                                                             bufs=2))
                rpool = at_scope.enter_context(tc.tile_pool(name="qrep",
                                                            bufs=2))
                spool = at_scope.enter_context(tc.tile_pool(name="ascr",
                                                            bufs=2))
                apool = at_scope.enter_context(tc.tile_pool(name="attn",
                                                            bufs=2))
                opool = at_scope.enter_context(tc.tile_pool(name="oev",
                                                            bufs=2))
                pp_pt = at_scope.enter_context(
                    tc.tile_pool(name="pppt", bufs=2, space="PSUM"))
                pp_sr = at_scope.enter_context(
                    tc.tile_pool(name="ppsr", bufs=2, space="PSUM"))
                pp_o = at_scope.enter_context(
                    tc.tile_pool(name="ppo", bufs=2, space="PSUM"))
                qT_rd = qT_dram.rearrange("m p s -> p m s")
                for s0 in range(NSUB):
                    csl = bass.ts(s0, SCH)
                    qTc = qcpool.tile([128, KO, SCH], mm, tag="qtc",
                                      name="qTc")
                    nc.sync.dma_start(qTc[:, :20, :], qT_rd[:, :20, csl])
                    nc.scalar.dma_start(qTc[:, 20:, :], qT_rd[:, 20:, csl])
                    qsc_rep = rpool.tile([128, SCH], F32, tag="qr",
                                         name="qsc_rep")
                    nc.gpsimd.partition_broadcast(
                        qsc_rep, qsc_h[s0 // 2][:, bass.ts(s0 % 2, SCH)])
                    for g in range(KO // 8):
                        nc.vector.tensor_mul(
                            qTc[:, bass.ts(g, 8), :], qTc[:, bass.ts(g, 8), :],
                            qsc_rep[:, None, :].to_broadcast([128, 8, SCH]))
                    for h in range(H):
                        pt = pp_pt.tile([128, LSUB, SCH], F32, tag="pt",
                                        name="pt")
                        for lb in range(LSUB):
                            nc.tensor.matmul(
                                pt[:, lb, :], kT_sb[:, h, bass.ts(lb, 128)],
                                qTc[:, h, :],
                                start=(lb % 2 == 0), stop=(lb % 2 == 1))
                        probsT = apool.tile([128, LSUB, SCH], mm,
                                            tag="probsT")
                        nc.scalar.activation(probsT, pt,
                                             mybir.ActivationFunctionType.Exp)
                        sr = pp_sr.tile([128, 512], F32, tag="sr", name="sr")
                        for lb in range(LSUB):
                            nc.tensor.matmul(sr[:1, :SCH], ones_sb,
                                             probsT[:, lb, :],
                                             start=(lb == 0),
                                             stop=(lb == LSUB - 1))
                        rinv = spool.tile([1, SCH], F32, tag="rinv",
                                          name="rinv")
                        nc.vector.reciprocal_approx_fast(rinv, sr[:1, :SCH])
                        ops = pp_o.tile([128, SCH], F32, tag="o", name="ops")
                        for lb in range(LSUB):
                            nc.tensor.matmul(ops, v_sb[:, lb, bass.ts(h, 128)],
                                             probsT[:, lb, :],
                                             start=(lb == 0),
                                             stop=(lb == LSUB - 1))
                        # replicate 1/sum across partitions off the PE
                        # stream (gpsimd is otherwise idle here)
                        rrep = spool.tile([128, SCH], F32, tag="rrep",
                                          name="rrep")
                        nc.gpsimd.partition_broadcast(rrep, rinv)
                        o_h = opool.tile([128, SCH], mm, tag="oh", name="o_h")
                        nc.vector.tensor_mul(o_h, ops, rrep)
                        nc.sync.dma_start(oT_r[:, h, bass.ts(s0, SCH)], o_h)

        # =========== output projection ===========
        with ExitStack() as ph:
            opool = ph.enter_context(tc.tile_pool(name="oT", bufs=1))
            wpool = ph.enter_context(tc.tile_pool(name="wo", bufs=2))
            spool = ph.enter_context(tc.tile_pool(name="oscr", bufs=3))
            pp_mm = ph.enter_context(tc.tile_pool(name="ppmmo", bufs=2,
                                                  space="PSUM"))

            oT_all = opool.tile([128, KO, S_SHARD], mm)
            bo_rep = opool.tile([128, D], mm, name="bo_rep")
            nc.gpsimd.dma_start(bo_rep, bot.ap()[None, :].to_broadcast([128, D]))
            # first wo tiles prefetch during the attention tail via SWDGE
            # (no deps), before the spill-gated oT readbacks enter the queue
            wo_tiles = []
            for t in range(2):
                wo_sb = wpool.tile([128, KO, 512], mm, tag="wo", name="wo_sb")
                nc.gpsimd.dma_start(
                    wo_sb, wo_p.ap()[t].rearrange("p (ko c) -> p ko c", ko=KO))
                wo_tiles.append(wo_sb)
            for c in range(NSUB):
                # scalar HWDGE is idle during attention; the last sub-chunk
                # (gated on the final spills) is split across both queues
                if c < NSUB - 1:
                    nc.scalar.dma_start(oT_all[:, :, bass.ts(c, SCH)],
                                        oT_r[:, :, bass.ts(c, SCH)])
                else:
                    csl = bass.ts(c, SCH)
                    nc.sync.dma_start(oT_all[:, :20, csl], oT_r[:, :20, csl])
                    nc.scalar.dma_start(oT_all[:, 20:, csl], oT_r[:, 20:, csl])
            for t in range(D // 512):
                if t < 2:
                    wo_sb = wo_tiles[t]
                else:
                    wo_sb = wpool.tile([128, KO, 512], mm, tag="wo",
                                       name="wo_sb")
                    wdma(t, wo_sb,
                         wo_p.ap()[t].rearrange("p (ko c) -> p ko c", ko=KO))
                for cs in range(S_SHARD // 128):
                    ps = pp_mm.tile([128, 512], F32, tag="mm", name="ps_o")
                    for ko in range(KO):
                        nc.tensor.matmul(ps, oT_all[:, ko, bass.ts(cs, 128)],
                                         wo_sb[:, ko, :],
                                         start=(ko == 0), stop=(ko == KO - 1))
                    o_sb = spool.tile([128, 512], mm, tag="out", name="o_sb")
                    nc.vector.tensor_add(o_sb, ps, bo_rep[:, bass.ts(t, 512)])
                    nc.scalar.dma_start(out_r[:, cs, bass.ts(t, 512)], o_sb)

    nc.compile()
    return nc


def _get_nc():
    global _CACHED_NC
    if _CACHED_NC is None:
        _CACHED_NC = _build()
    return _CACHED_NC


def _pack_w(wT, tc):
    """[D, N] (contraction-major transposed weight) -> [N//tc, 128, KO*tc]
    so each streamed tile is one fully-contiguous DMA read."""
    n = wT.shape[1]
    nt = n // tc
    return np.ascontiguousarray(
        wT.reshape(KO, 128, nt, tc).transpose(2, 1, 0, 3).reshape(
            nt, 128, KO * tc))


def kernel(hidden_cond, hidden_uncond, context_cond, context_uncond,
           Wq, bq, Wkv, bkv, gq, gk, Wo, bo):
    global LAST_EXEC_NS
    import ml_dtypes
    bf = ml_dtypes.bfloat16 if MM == mybir.dt.bfloat16 else np.float32
    f32 = np.float32

    nc = _get_nc()

    hid = [np.asarray(hidden_cond, f32).reshape(-1, D),
           np.asarray(hidden_uncond, f32).reshape(-1, D)]
    ctxs = [np.asarray(context_cond, f32).reshape(-1, D),
            np.asarray(context_uncond, f32).reshape(-1, D)]
    Wq = np.asarray(Wq, f32)
    Wkv = np.asarray(Wkv, f32)
    Wo = np.asarray(Wo, f32)
    bq = np.asarray(bq, f32)
    bkv = np.asarray(bkv, f32)
    bo = np.asarray(bo, f32)
    gq = np.asarray(gq, f32)
    gk = np.asarray(gk, f32)
    bk, bv = bkv[:D], bkv[D:]

    wq_pk = _pack_w(np.ascontiguousarray(Wq.T).astype(bf), 128)
    wo_pk = _pack_w(np.ascontiguousarray(Wo.T).astype(bf), 512)
    WkT = np.ascontiguousarray(Wkv[:D].T).astype(bf)
    WvT = np.ascontiguousarray(Wkv[D:].T).astype(bf)
    wk_pks = [_pack_w(WkT[:, r * VSH:(r + 1) * VSH], 256) for r in range(R)]
    wv_pks = [_pack_w(WvT[:, r * VSH:(r + 1) * VSH], 256) for r in range(R)]

    common = {
        "wq_p": wq_pk, "wo_p": wo_pk,
        "gq_pm": np.ascontiguousarray(gq.reshape(KO, 128).T),
        "bqgq_pm": np.ascontiguousarray((bq * gq).reshape(KO, 128).T),
        "bo": bo,
    }
    cT_ps = []
    for g in range(2):
        cT = np.ascontiguousarray(ctxs[g].T).astype(bf)   # [D, L]
        cT_ps.append(np.ascontiguousarray(
            cT.reshape(KO, 128, L).transpose(1, 0, 2).reshape(128, KO * L)))

    in_maps = []
    for core in range(8):
        g, r = core // 4, core % 4
        hT = np.ascontiguousarray(
            hid[g][r * S_SHARD:(r + 1) * S_SHARD].T).astype(bf)  # [D, S_SHARD]
        hT_pk = np.ascontiguousarray(
            hT.reshape(KO, 128, S_SHARD).transpose(1, 0, 2)
            .reshape(128, KO * S_SHARD))
        sl = slice(r * VSH, (r + 1) * VSH)
        in_maps.append({
            "hT_p": hT_pk, "cT_p": cT_ps[g],
            "wk_p": wk_pks[r], "wv_p": wv_pks[r],
            "gk_pm": np.ascontiguousarray(gk[sl].reshape(MSH, 128).T),
            "bkgk_pm": np.ascontiguousarray((bk * gk)[sl].reshape(MSH, 128).T),
            "bv_sh": np.ascontiguousarray(bv[sl]),
            **common,
        })

    res = bass_utils.run_bass_kernel_spmd(nc, in_maps, list(range(8)),
                                          trace=TRACE)
    LAST_EXEC_NS = res.exec_time_ns

    out_c = np.concatenate(
        [np.asarray(res.results[i]["out"], f32) for i in range(4)], axis=0)
    out_u = np.concatenate(
        [np.asarray(res.results[i]["out"], f32) for i in range(4, 8)], axis=0)
    return (out_c[None], out_u[None])
